# revision 86
# baseline (speedup 1.0000x reference)
"""Trainium2 Bass kernel for NanoAttention (B=4, T=2048, C=1024, H=16, causal).

Sharding: 8 cores = 4 batches x 2 head-groups (8 heads each).

v2: list-scheduled emission. The attention inner loop is ACT-bound (the
softmax exp on the Scalar engine costs ~2x the S+AV matmul PE time), so the
builder runs a clock-tracking greedy scheduler that interleaves qkv / proj
matmul blocks into the exp-latency gaps, keeps the PE continuously busy (which
also keeps it in the fast p-state), and starts the first matmul as soon as the
first x^T/W chunks land instead of after all weights.

Engine division of labor:
  PE     - all matmuls (qkv, S, AV, proj, tail reciprocal-broadcast)
  ACT    - only the softmax exp (the bottleneck op, nothing else)
  DVE    - PSUM->SBUF casts, softmax denominators via reciprocal_approx_fast
           (replaces the baseline's Ln+Exp ACT chain), normalize muls
  Pool   - causal tri-mask muls (SBUF-only, otherwise idle engine)
  DMA    - loads (priority-ordered), broadcast of 1/denom rows, bf16 stores
"""
import os
import sys

sys.path.insert(0, '/opt/trn_rl_repo')

import numpy as np
import orjson

import concourse.bass as bass
import concourse.mybir as mybir
import concourse.tile as tile
from concourse.bass_utils import run_bass_kernel_spmd

# ---------------------------------------------------------------------------
# Workaround for this container's walrus build: it enforces the HW limit of
# one sync-wait per instruction (two for EventSemaphore), but Tile's sem
# assignment can emit more (kernel-tail Drain waits on every DMA queue used;
# HWDGE stores can pick up two queue waits). Split the overflow onto
# preceding pure-wait EventSemaphore instructions on the same engine at
# JSON-serialization time so every compile path is covered.
# ---------------------------------------------------------------------------


def _split_multi_waits(data):
    n_split = 0
    for func in data.get("functions", []):
        for blk in func.get("blocks", []):
            insts = blk.get("instructions")
            if not insts:
                continue
            out = []
            for inst in insts:
                si = inst.get("sync_info")
                waits = (si or {}).get("on_wait") or []
                cap = 2 if inst.get("opcode") == "EventSemaphore" else 1
                if len(waits) > cap and "engine" in inst:
                    extra = waits[:-cap]
                    si["on_wait"] = waits[-cap:]
                    for i in range(0, len(extra), 2):
                        n_split += 1
                        out.append({
                            "debug": inst.get("debug"),
                            "engine": inst["engine"],
                            "ins": [],
                            "outs": [],
                            "name": f"{inst['name']}_wsplit{n_split}",
                            "opcode": "EventSemaphore",
                            "sync_info": {"on_wait": extra[i:i + 2],
                                          "on_update": []},
                        })
                out.append(inst)
            blk["instructions"] = out
    return data


_orig_to_json_bytes = bass.Bass.to_json_bytes


def _patched_to_json_bytes(self):
    return orjson.dumps(_split_multi_waits(orjson.loads(_orig_to_json_bytes(self))))


bass.Bass.to_json_bytes = _patched_to_json_bytes

# ---------------------------------------------------------------------------

B, T, C = 4, 2048, 1024
N_HEAD, D = 16, 64
HLOC = 8          # heads per core
CLOC = HLOC * D   # 512 local qkv channels per core
QG = 512          # query-group width
NG = T // QG      # 4 query groups
KB = 128          # key-block width
F32R = mybir.dt.float32r
F32 = mybir.dt.float32
BF16 = mybir.dt.bfloat16
EXP = mybir.ActivationFunctionType.Exp
SCALE = 1.0 / np.sqrt(D)
TRI_POOL = os.environ.get("ATTN_TRI", "pool") == "pool"
RECIP_DVE = os.environ.get("ATTN_RECIP", "dve") == "dve"

# scheduler clock model (ns)
PE_COL = 0.43          # per matmul output column, warm clock
ACT_EL = 0.833         # per exp element (per partition-lane)
ACT_FIX = 260.0        # per exp instruction overhead
SEM = 180.0            # cross-engine sem propagation
CAST_LAT = 950.0       # PSUM->SBUF cast completing after producer matmul
TRI_LAT = 650.0        # pool tri-mask mul latency after exp
NORM_LAT = 1600.0      # psy release after last AV (one DVE cast)
LEAD = 6000.0          # how far ACT may run ahead of PE before S throttles
KEEP = 1500.0          # ACT backlog above which fillers take priority over S
DMA_BW = 0.0033        # ns per byte (~300 GB/s effective)
DMA_LAT = 1900.0       # DGE issue + first-byte latency


def _build_body(nc, tc, ctx, xt, wqkt, wvt, wpt, tri, ot):
    p_wqk = ctx.enter_context(tc.tile_pool(name="wqk", bufs=8))
    p_wv = ctx.enter_context(tc.tile_pool(name="wv", bufs=8))
    p_wp = ctx.enter_context(tc.tile_pool(name="wp", bufs=4))
    p_xt = ctx.enter_context(tc.tile_pool(name="xt", bufs=16))
    p_k = ctx.enter_context(tc.tile_pool(name="ksb", bufs=4))
    p_q = ctx.enter_context(tc.tile_pool(name="qsb", bufs=8))
    p_vp = ctx.enter_context(tc.tile_pool(name="vp", bufs=16))
    p_es = ctx.enter_context(tc.tile_pool(name="es", bufs=16))
    p_ysb = ctx.enter_context(tc.tile_pool(name="ysb", bufs=5))
    p_rec = ctx.enter_context(tc.tile_pool(name="rec", bufs=1))
    p_bc = ctx.enter_context(tc.tile_pool(name="bc", bufs=8))
    p_yt = ctx.enter_context(tc.tile_pool(name="yt", bufs=16))
    p_ost = ctx.enter_context(tc.tile_pool(name="ost", bufs=4))
    p_one = ctx.enter_context(tc.tile_pool(name="one", bufs=1))
    p_drb = ctx.enter_context(tc.tile_pool(name="drb", bufs=4, space="DRAM"))
    ps_mm = ctx.enter_context(tc.tile_pool(name="psmm", bufs=2, space="PSUM"))
    ps_s = ctx.enter_context(tc.tile_pool(name="pss", bufs=2, space="PSUM"))
    ps_y = ctx.enter_context(tc.tile_pool(name="psy", bufs=1, space="PSUM"))

    # ---------------- static tensors / initial DMA batch ----------------
    dma_t = 0.0

    def dma_est(nbytes):
        nonlocal dma_t
        dma_t += nbytes * DMA_BW
        return dma_t + DMA_LAT

    wqk_sb = [None] * 8
    wv_sb = [None] * 8
    wp_sb = [None] * 4
    xt_sb = {}
    xt_ready = {}
    wqk_ready = [0.0] * 8

    def load_xt(g):
        for kc in range(8):
            t = p_xt.tile([128, QG], BF16, tag="xt", name=f"xt{g}_{kc}")
            nc.sync.dma_start(out=t, in_=xt[kc * 128:(kc + 1) * 128,
                                           g * QG:(g + 1) * QG])
            xt_sb[(g, kc)] = t
            xt_ready[(g, kc)] = dma_est(128 * QG * 2)

    # interleave x(g0) and W_qk chunks so the first qkv m-block can start
    # after one pair instead of after the full weight load; spread the issue
    # across three HWDGE queues (SP/DVE/ACT) since startup is descriptor-
    # generation limited on a single queue
    for kc in range(8):
        t = p_xt.tile([128, QG], BF16, tag="xt", name=f"xt0_{kc}")
        nc.sync.dma_start(out=t, in_=xt[kc * 128:(kc + 1) * 128, 0:QG])
        xt_sb[(0, kc)] = t
        xt_ready[(0, kc)] = dma_est(128 * QG * 2)
        w = p_wqk.tile([128, 2 * CLOC], BF16, tag="wqk", name=f"wqk{kc}")
        nc.sync.dma_start(out=w, in_=wqkt[kc * 128:(kc + 1) * 128, :])
        wqk_sb[kc] = w
        wqk_ready[kc] = dma_est(128 * 2 * CLOC * 2)
    wv_ready = [0.0] * 8
    for kc in range(8):
        w = p_wv.tile([128, CLOC], BF16, tag="wv", name=f"wv{kc}")
        nc.sync.dma_start(out=w, in_=wvt[kc * 128:(kc + 1) * 128, :])
        wv_sb[kc] = w
        wv_ready[kc] = dma_est(128 * CLOC * 2)
    tri2 = p_one.tile([128, 2, KB], BF16, tag="tri2")
    nc.sync.dma_start(out=tri2[:, 0, :], in_=tri[:, :])
    nc.sync.dma_start(out=tri2[:, 1, :], in_=tri[:, :])
    dma_est(2 * KB * KB * 2)
    ones_sb = p_one.tile([1, 64], F32R, tag="ones")
    nc.vector.memset(ones_sb.bitcast(F32), 1.0)
    # preload the Exp table during the initial DMA wait
    scr = p_one.tile([1, 64], F32, tag="scr")
    nc.scalar.activation(out=scr, in_=ones_sb.bitcast(F32), func=EXP, scale=1.0)

    def load_wp():
        for c in range(4):
            w = p_wp.tile([128, C], BF16, tag="wp", name=f"wp{c}")
            nc.sync.dma_start(out=w, in_=wpt[c * 128:(c + 1) * 128, :])
            wp_sb[c] = w
            dma_est(128 * C * 2)

    # persistent attention tensors
    k_sb = [p_k.tile([128, T], BF16, tag="ksb", name=f"ksb{c}") for c in range(4)]
    vp_sb = [None] * 16
    q_sb = {}
    yt_sb = {g: None for g in range(NG)}

    # ---------------- emission helpers ----------------
    pe_t = 0.0
    act_t = 0.0
    q_ready = {}
    v_ready = {}

    def emit_mm_block(g, m, is_v):
        """one qkv m-block: 8 accumulating matmuls + PSUM->SBUF cast"""
        nonlocal pe_t
        ps = ps_mm.tile([128, QG], F32, tag="psmm", name=f"mm{g}_{m}_{is_v}")
        for kc in range(8):
            if is_v:
                nc.tensor.matmul(ps, xt_sb[(g, kc)][:, m * 128:(m + 1) * 128],
                                 wv_sb[kc], start=kc == 0, stop=kc == 7,
                                 skip_group_check=True)
                rdy = max(xt_ready[(g, kc)], wv_ready[kc])
            else:
                nc.tensor.matmul(ps, wqk_sb[kc][:, m * 128:(m + 1) * 128],
                                 xt_sb[(g, kc)], start=kc == 0, stop=kc == 7,
                                 skip_group_check=True)
                rdy = max(xt_ready[(g, kc)], wqk_ready[kc])
            pe_t = max(pe_t + QG * PE_COL, rdy + QG * PE_COL)
        if is_v:
            vp = p_vp.tile([128, HLOC, 65], BF16, tag="vp", name=f"vp{g}_{m}")
            nc.vector.memset(vp[:, :, 64:65], 1.0)
            nc.vector.tensor_copy(out=vp[:, :, 0:64],
                                  in_=ps.rearrange("p (h d) -> p h d", d=64))
            vp_sb[g * 4 + m] = vp
            v_ready[g * 4 + m] = pe_t + CAST_LAT
        elif m >= 4:
            nc.vector.tensor_copy(
                out=k_sb[m - 4][:, g * QG:(g + 1) * QG], in_=ps)
        else:
            qt = p_q.tile([128, QG], BF16, tag="qsb", name=f"q{g}_{m}")
            nc.vector.tensor_copy(out=qt, in_=ps)
            q_sb[(g, m)] = qt
            q_ready[(g, m)] = pe_t + CAST_LAT

    def emit_S(u):
        nonlocal pe_t, act_t
        g, hp, kb = u["g"], u["hp"], u["kb"]
        c0 = u["c0"]
        vis = slice(c0, QG)
        w = QG - c0
        ps = ps_s.tile([128, 2, QG], F32, tag="pss", name=f"s{g}_{hp}_{kb}")
        for r in (0, 1):
            row = slice(64 * r, 64 * r + 64)
            nc.tensor.matmul(ps[:, r, vis],
                             k_sb[hp][row, kb * 128:(kb + 1) * 128],
                             q_sb[(g, hp)][row, vis], start=True, stop=True,
                             skip_group_check=True)
        pe_t = max(pe_t, u["rdy"]) + 2 * w * PE_COL
        es = p_es.tile([128, 2, QG], BF16, tag="es", name=f"e{g}_{hp}_{kb}")
        nc.scalar.activation(out=es[:, :, vis], in_=ps[:, :, vis],
                             func=EXP, scale=SCALE)
        act_t = max(act_t, pe_t + SEM) + 2 * w * ACT_EL + ACT_FIX
        u["es_est"] = act_t + SEM
        if kb >= 4 * g:  # diagonal block: causal tri mask (on idle Pool
            # engine; DVE for the last hp where the tri latency is critical)
            last_hp = g == NG - 1 and hp == 3
            eng = nc.vector if (last_hp or not TRI_POOL) else nc.gpsimd
            eng.tensor_mul(es[:, :, c0:c0 + 128], es[:, :, c0:c0 + 128], tri2)
            u["es_est"] += 250.0 if last_hp else TRI_LAT
        u["es"] = es
        u["exp_done"] = act_t

    def emit_AV(u, psy, k_last):
        nonlocal pe_t
        g, hp, kb = u["g"], u["hp"], u["kb"]
        vis = slice(u["c0"], QG)
        w = QG - u["c0"]
        for r in (0, 1):
            nc.tensor.matmul(psy[0:65, r, vis], vp_sb[kb][:, 2 * hp + r, :],
                             u["es"][:, r, vis], start=kb == 0,
                             stop=k_last, skip_group_check=True)
        pe_t = max(pe_t, u["av_rdy"]) + 2 * w * PE_COL

    gden = {}
    gysb = {}
    pending_muls = []
    AOP = mybir.AluOpType

    def pop_mul():
        yt, r, ysb, bc = pending_muls.pop(0)
        nc.vector.scalar_tensor_tensor(
            out=yt[64 * r:64 * r + 64, :], in0=ysb[0:64, r, :],
            scalar=-1.0, in1=bc, op0=AOP.mult, op1=AOP.mult)

    def emit_norm(g, hp, psy, pe_bcast):
        """softmax denominators + normalize: PSUM psy -> SBUF yt (bf16).
        Non-tail groups batch 1/denominator as a Newton-seed reciprocal on
        DVE, once per group, keeping the saturated ACT engine exp-only."""
        nonlocal pe_t, act_t
        if yt_sb[g] is None:
            yt_sb[g] = [None] * 4
        ysb = p_ysb.tile([65, 2, QG], F32, tag="ysb", name=f"yb{g}_{hp}")
        if pe_bcast:
            # kernel tail: batched Ln+Exp straight off the PSUM denominator
            # rows; the broadcast matmul then reads the rec rows directly
            yt = p_yt.tile([128, QG], BF16, tag="yt", name=f"yt{g}_{hp}")
            yt_sb[g][hp] = yt
            ln2 = p_one.tile([1, 2, QG], F32, tag="ln2", name=f"ln2{g}_{hp}")
            nc.scalar.activation(out=ln2, in_=psy[64:65, :, :],
                                 func=mybir.ActivationFunctionType.Ln)
            rec2 = p_one.tile([1, 2, QG], F32R, tag="rec2", name=f"rc2{g}_{hp}")
            nc.scalar.activation(out=rec2, in_=ln2, func=EXP, scale=-1.0)
            nc.vector.tensor_copy(out=ysb, in_=psy[0:65, :, :])
            psb = ps_y.tile([128, 2, QG], F32, tag="psy", name=f"pb{g}_{hp}")
            for r in (0, 1):
                nc.tensor.matmul(psb[0:64, r, :], ones_sb, rec2[:, r, :],
                                 start=True, stop=True, skip_group_check=True)
                pe_t += QG * PE_COL
                nc.vector.tensor_mul(yt[64 * r:64 * r + 64, :],
                                     ysb[0:64, r, :], psb[0:64, r, :])
            return
        # single cast frees the PSUM psy tile fast (next hp's AVs wait on
        # it); on ACT ('copy' is in-table) since DVE may be busy with the
        # qkv-pool casts at hp boundaries
        nc.scalar.activation(out=ysb, in_=psy[0:65, :, :],
                             func=mybir.ActivationFunctionType.Copy)
        if g not in gden:
            gden[g] = p_rec.tile([8, QG], F32, tag="gd", name=f"gd{g}")
            gysb[g] = {}
        gysb[g][hp] = ysb
        # gather both denominator rows with one small DMA (engine writes
        # must start at partition 0/32/64/96; DMA has no such restriction)
        nc.sync.dma_start(out=gden[g][2 * hp:2 * hp + 2, :],
                          in_=ysb[64:65, :, :])
        last_hp = 2 if g == NG - 1 else 3
        if hp != last_hp:
            return
        # one Newton-reciprocal chain for the whole group's 6-8 denominators
        # (sign-carried: y2 = -1/d; the final muls fold in the -1)
        d = gden[g]
        nb = p_rec.tile([8, QG], F32, tag="nb", name=f"nb{g}")
        I32 = mybir.dt.int32
        nc.vector.tensor_tensor(out=nb.bitcast(I32), in0=d.bitcast(I32),
                                in1=d.bitcast(I32), op=AOP.bitwise_not)
        y0 = p_rec.tile([8, QG], F32, tag="y0", name=f"y0{g}")
        nc.vector.tensor_scalar_mul(out=y0, in0=nb, scalar1=0.23549792)
        t1 = p_rec.tile([8, QG], F32, tag="t1", name=f"t1{g}")
        nc.vector.tensor_mul(t1, d, y0)
        y1 = p_rec.tile([8, QG], F32, tag="y1", name=f"y1{g}")
        nc.vector.scalar_tensor_tensor(out=y1, in0=t1, scalar=2.0017324,
                                       in1=y0, op0=AOP.add, op1=AOP.mult)
        t2 = p_rec.tile([8, QG], F32, tag="t2", name=f"t2{g}")
        nc.vector.tensor_mul(t2, d, y1)
        y2 = p_rec.tile([8, QG], F32, tag="y2", name=f"y2{g}")
        nc.vector.scalar_tensor_tensor(out=y2, in0=t2, scalar=2.0,
                                       in1=y1, op0=AOP.add, op1=AOP.mult)
        drec = p_drb.tile([8, QG], F32, tag="drec", name=f"dr{g}")
        nc.sync.dma_start(out=drec, in_=y2)
        for hq in range(last_hp + 1):
            yt = p_yt.tile([128, QG], BF16, tag="yt", name=f"yt{g}_{hq}")
            yt_sb[g][hq] = yt
            for r in (0, 1):
                j = 2 * hq + r
                bc = p_bc.tile([64, QG], F32, tag="bc", name=f"bc{g}_{j}")
                nc.sync.dma_start(
                    out=bc, in_=drec[j:j + 1, :].to_broadcast([64, QG]))
                # defer the mul: the scheduler spreads these through the DVE
                # stream so the 8-op burst doesn't delay qkv/proj PSUM casts
                pending_muls.append((yt, r, gysb[g][hq], bc))

    p3_tiles = {}

    def emit_proj(g, m, c_lo=0, ps=None):
        nonlocal pe_t
        if ps is None:
            ps = ps_mm.tile([128, QG], F32, tag="psmm", name=f"pj{g}_{m}")
        for c in range(c_lo, 4):
            nc.tensor.matmul(ps, wp_sb[c][:, m * 128:(m + 1) * 128],
                             yt_sb[g][c], start=c == 0, stop=c == 3,
                             skip_group_check=True)
        pe_t += (4 - c_lo) * QG * PE_COL
        ost = p_ost.tile([128, QG], BF16, tag="ost", name=f"o{g}_{m}")
        nc.vector.tensor_copy(out=ost, in_=ps)
        # final group's stores go out on the (idle-by-then) ACT queue so the
        # kernel tail doesn't wait behind the SP queue's issue backlog
        eng = nc.scalar if g == NG - 1 else nc.sync
        eng.dma_start(out=ot[m * 128:(m + 1) * 128, g * QG:(g + 1) * QG],
                      in_=ost)

    def emit_proj3_partial(m, ps=None):
        # first 3 contraction chunks of a final-group proj block; the last
        # chunk + store happen in the tail once hp3's yt lands
        nonlocal pe_t
        if ps is None:
            ps = ps_mm.tile([128, QG], F32, tag="psmm", name=f"pj3p_{m}")
        for c in range(3):
            nc.tensor.matmul(ps, wp_sb[c][:, m * 128:(m + 1) * 128],
                             yt_sb[3][c], start=c == 0, stop=False,
                             skip_group_check=True)
        pe_t += 3 * QG * PE_COL
        p3_tiles[m] = ps

    def emit_proj3_partial2(m0):
        # a pair of partial blocks sharing one (by-then idle) S-pool tile
        ps2 = ps_s.tile([128, 2, QG], F32, tag="pss", name=f"pj3q_{m0}")
        emit_proj3_partial(m0, ps2[:, 0, :])
        emit_proj3_partial(m0 + 1, ps2[:, 1, :])

    # ---------------- unit and filler lists ----------------
    units = []
    for g in range(NG):
        for hp in range(4):
            for kb in range(4 * (g + 1)):
                units.append({"g": g, "hp": hp, "kb": kb,
                              "c0": max(0, 128 * (kb - 4 * g))})
    fillers = []
    for g in range(NG):
        if g > 0:
            fillers.append(("xt", g))
        if g == 1:
            fillers.append(("wp",))
        for hp in range(4):
            fillers.append(("kq", g, 4 + hp))  # k chunk
            fillers.append(("kq", g, hp))      # q chunk
        for tb in range(4):
            fillers.append(("v", g, tb))
    for g in range(NG - 1):
        for m in range(8):
            fillers.append(("proj", g, m))

    # ---------------- greedy clock-driven scheduler ----------------
    s_idx = 0
    f_idx = 0
    av_units = []          # exp-emitted units awaiting AV, lex order
    exp_done_hist = []     # S psum recycle tracking (pool depth 2)
    psy_free_est = 0.0
    yt_ready = {}
    cur_psy = None
    cur_av_key = None      # (g, hp) whose AVs are in flight

    def s_deps(u):
        gk = u["kb"] // 4
        qr = q_ready.get((u["g"], u["hp"]))
        if qr is None or (gk, u["hp"]) not in k_emitted:
            return None
        return max(qr, k_ready.get((gk, u["hp"]), 0.0))

    k_emitted = set()
    k_ready = {}

    def filler_ok(f):
        if f[0] == "proj":
            return f[1] in yt_ready
        if f[0] in ("p3a", "p3b"):
            # pure-tail fill: emit only after the last AV so the partial
            # matmuls don't push the critical-path AVs back in the PE queue
            # (p3b also takes an S-pool tile, unsafe while S units remain)
            return s_idx >= len(units) and not av_units
        return True

    def run_filler(f):
        nonlocal pe_t
        if f[0] == "xt":
            load_xt(f[1])
        elif f[0] == "wp":
            load_wp()
        elif f[0] == "kq":
            g, m = f[1], f[2]
            emit_mm_block(g, m, False)
            if m >= 4:
                k_emitted.add((g, m - 4))
                k_ready[(g, m - 4)] = pe_t + CAST_LAT
        elif f[0] == "v":
            emit_mm_block(f[1], f[2], True)
        elif f[0] == "p3a":
            emit_proj3_partial(f[1])
        elif f[0] == "p3b":
            emit_proj3_partial2(f[1])
        elif f[0] == "proj":
            pe_t = max(pe_t, yt_ready[f[1]])
            emit_proj(f[1], f[2])

    def do_av(u, forced):
        nonlocal pe_t, psy_free_est, cur_av_key, cur_psy
        key = (u["g"], u["hp"])
        if cur_av_key is None:
            if forced:
                pe_t = max(pe_t, psy_free_est)
            cur_psy = ps_y.tile([128, 2, QG], F32, tag="psy",
                                name=f"py{u['g']}_{u['hp']}")
            cur_av_key = key
        av_units.pop(0)
        u["av_rdy"] = max(u["es_est"], v_ready.get(u["kb"], 0.0))
        k_last = u["kb"] == 4 * (u["g"] + 1) - 1
        emit_AV(u, cur_psy, k_last)
        if k_last:
            g, hp = key
            pe_bcast = g == NG - 1 and hp == 3
            emit_norm(g, hp, cur_psy, pe_bcast)
            psy_free_est = pe_t + NORM_LAT
            if hp == 3:
                yt_ready[g] = pe_t + NORM_LAT + 600.0
            if g == NG - 1 and hp == 2:
                fillers.append(("p3a", 0))
                fillers.append(("p3a", 1))
                fillers.append(("p3b", 2))
                fillers.append(("p3b", 4))
            cur_av_key = None
            cur_psy = None

    def try_S():
        nonlocal s_idx
        u = units[s_idx]
        rdy = s_deps(u)
        depth_ok = (len(exp_done_hist) < 2
                    or exp_done_hist[-2] <= pe_t + 250)
        if (rdy is not None and rdy <= pe_t + 250
                and act_t <= pe_t + LEAD and depth_ok
                and len(av_units) < 15):
            u["rdy"] = rdy
            emit_S(u)
            exp_done_hist.append(u["exp_done"])
            av_units.append(u)
            s_idx += 1
            return True
        return False

    while s_idx < len(units) or av_units or f_idx < len(fillers):
        if pending_muls:
            pop_mul()
        # 1) AV whose es is (estimated) ready
        if av_units:
            u = av_units[0]
            key = (u["g"], u["hp"])
            ok = (u["es_est"] <= pe_t + 60
                  and v_ready.get(u["kb"], 1e18) <= pe_t + 60)
            if ok and cur_av_key is None:
                ok = psy_free_est <= pe_t + 60
            if ok and (cur_av_key is None or cur_av_key == key):
                do_av(u, False)
                continue
        # 2) when ACT already has a healthy backlog, race qkv fillers forward
        # (kq blocks unlock the NEXT group's exp work - emitting them early
        # lets attention pull forward across window boundaries; proj blocks
        # don't enable anything, keep them in reserve for exp-bound gaps)
        act_healthy = act_t > pe_t + KEEP
        if (act_healthy and f_idx < len(fillers)
                and fillers[f_idx][0] != "proj"
                and filler_ok(fillers[f_idx])):
            run_filler(fillers[f_idx])
            f_idx += 1
            continue
        # 3) S unit if deps ready and ACT not over-backlogged
        if s_idx < len(units) and try_S():
            continue
        # 4) filler
        if f_idx < len(fillers) and filler_ok(fillers[f_idx]):
            run_filler(fillers[f_idx])
            f_idx += 1
            continue
        # 5) forced progress (stall): prefer AV, then S, then proj
        if av_units:
            u = av_units[0]
            key = (u["g"], u["hp"])
            if cur_av_key is None or cur_av_key == key:
                do_av(u, True)
                continue
        if s_idx < len(units):
            u = units[s_idx]
            rdy = s_deps(u)
            if rdy is not None:
                pe_t = max(pe_t, rdy)
                if len(exp_done_hist) >= 2:
                    pe_t = max(pe_t, exp_done_hist[-2])
                u["rdy"] = rdy
                emit_S(u)
                exp_done_hist.append(u["exp_done"])
                av_units.append(u)
                s_idx += 1
                continue
        if f_idx < len(fillers):
            f = fillers[f_idx]
            if f[0] == "proj":
                pe_t = max(pe_t, yt_ready.get(f[1], pe_t))
            run_filler(f)
            f_idx += 1
            continue
        raise RuntimeError("scheduler wedged")

    while pending_muls:
        pop_mul()
    # tail: final group's proj (finish the pre-accumulated blocks first)
    for m in range(8):
        if m in p3_tiles:
            emit_proj(NG - 1, m, c_lo=3, ps=p3_tiles[m])
        else:
            emit_proj(NG - 1, m)


def _build_nc():
    from contextlib import ExitStack
    nc = bass.Bass(trn_type="TRN2")
    xt = nc.dram_tensor("xt", [C, T], BF16, kind="ExternalInput")
    wqkt = nc.dram_tensor("wqkt", [C, 2 * CLOC], BF16, kind="ExternalInput")
    wvt = nc.dram_tensor("wvt", [C, CLOC], BF16, kind="ExternalInput")
    wpt = nc.dram_tensor("wpt", [CLOC, C], BF16, kind="ExternalInput")
    tri = nc.dram_tensor("tri", [KB, KB], BF16, kind="ExternalInput")
    ot = nc.dram_tensor("ot", [C, T], BF16, kind="ExternalOutput")
    with tile.TileContext(nc) as tc:
        with ExitStack() as ctx:
            _build_body(nc, tc, ctx, xt, wqkt, wvt, wpt, tri, ot)
    return nc


LAST_RESULTS = None
_NC_CACHE = None


def kernel(x, W_qkv, W_proj):
    global LAST_RESULTS, _NC_CACHE
    import ml_dtypes
    x = np.asarray(x, dtype=np.float32)
    W_qkv = np.asarray(W_qkv, dtype=np.float32)
    W_proj = np.asarray(W_proj, dtype=np.float32)

    if _NC_CACHE is None:
        _NC_CACHE = _build_nc()
    nc = _NC_CACHE
    _conv = lambda a: a.astype(ml_dtypes.bfloat16)
    tri = np.ascontiguousarray(np.triu(np.ones((KB, KB), np.float32)))
    in_maps = []
    for core in range(8):
        b, hg = core // 2, core % 2
        rq = slice(CLOC * hg, CLOC * hg + CLOC)
        Wq = W_qkv[0:C][rq]
        Wk = W_qkv[C:2 * C][rq]
        Wv = W_qkv[2 * C:3 * C][rq]
        in_maps.append({
            "xt": _conv(np.ascontiguousarray(x[b].T)),
            "wqkt": _conv(np.ascontiguousarray(np.concatenate([Wq, Wk], axis=0).T)),
            "wvt": _conv(np.ascontiguousarray(Wv.T)),
            "wpt": _conv(np.ascontiguousarray(W_proj[:, rq].T)),
            "tri": _conv(tri),
        })

    trace = os.environ.get("ATTN_BASS_TRACE") == "1"
    res = None
    last_exc = None
    for attempt in range(3):
        try:
            res = run_bass_kernel_spmd(nc, in_maps, core_ids=list(range(8)),
                                       trace=trace)
            break
        except Exception as e:  # transient NRT device errors happen
            last_exc = e
            import time as _time
            _time.sleep(2.0)
    if res is None:
        raise last_exc
    LAST_RESULTS = res
    out = np.empty((B, T, C), np.float32)
    for b in range(B):
        out[b] = (res.results[2 * b]["ot"].astype(np.float32)
                  + res.results[2 * b + 1]["ot"].astype(np.float32)).T
    return out


# revision 90
# speedup vs baseline: 1.0126x; 1.0126x over previous
"""Trainium2 Bass kernel for NanoAttention (B=4, T=2048, C=1024, H=16, causal).

Sharding: 8 cores = 4 batches x 2 head-groups (8 heads each).

v2: list-scheduled emission. The attention inner loop is ACT-bound (the
softmax exp on the Scalar engine costs ~2x the S+AV matmul PE time), so the
builder runs a clock-tracking greedy scheduler that interleaves qkv / proj
matmul blocks into the exp-latency gaps, keeps the PE continuously busy (which
also keeps it in the fast p-state), and starts the first matmul as soon as the
first x^T/W chunks land instead of after all weights.

Engine division of labor:
  PE     - all matmuls (qkv, S, AV, proj, tail reciprocal-broadcast)
  ACT    - only the softmax exp (the bottleneck op, nothing else)
  DVE    - PSUM->SBUF casts, softmax denominators via reciprocal_approx_fast
           (replaces the baseline's Ln+Exp ACT chain), normalize muls
  Pool   - causal tri-mask muls (SBUF-only, otherwise idle engine)
  DMA    - loads (priority-ordered), broadcast of 1/denom rows, bf16 stores
"""
import os
import sys

sys.path.insert(0, '/opt/trn_rl_repo')

import numpy as np
import orjson

import concourse.bass as bass
import concourse.mybir as mybir
import concourse.tile as tile
from concourse.bass_utils import run_bass_kernel_spmd

# ---------------------------------------------------------------------------
# Workaround for this container's walrus build: it enforces the HW limit of
# one sync-wait per instruction (two for EventSemaphore), but Tile's sem
# assignment can emit more (kernel-tail Drain waits on every DMA queue used;
# HWDGE stores can pick up two queue waits). Split the overflow onto
# preceding pure-wait EventSemaphore instructions on the same engine at
# JSON-serialization time so every compile path is covered.
# ---------------------------------------------------------------------------


def _split_multi_waits(data):
    n_split = 0
    for func in data.get("functions", []):
        for blk in func.get("blocks", []):
            insts = blk.get("instructions")
            if not insts:
                continue
            out = []
            for inst in insts:
                si = inst.get("sync_info")
                waits = (si or {}).get("on_wait") or []
                cap = 2 if inst.get("opcode") == "EventSemaphore" else 1
                if len(waits) > cap and "engine" in inst:
                    extra = waits[:-cap]
                    si["on_wait"] = waits[-cap:]
                    for i in range(0, len(extra), 2):
                        n_split += 1
                        out.append({
                            "debug": inst.get("debug"),
                            "engine": inst["engine"],
                            "ins": [],
                            "outs": [],
                            "name": f"{inst['name']}_wsplit{n_split}",
                            "opcode": "EventSemaphore",
                            "sync_info": {"on_wait": extra[i:i + 2],
                                          "on_update": []},
                        })
                out.append(inst)
            blk["instructions"] = out
    return data


_orig_to_json_bytes = bass.Bass.to_json_bytes


def _patched_to_json_bytes(self):
    return orjson.dumps(_split_multi_waits(orjson.loads(_orig_to_json_bytes(self))))


bass.Bass.to_json_bytes = _patched_to_json_bytes

# ---------------------------------------------------------------------------

B, T, C = 4, 2048, 1024
N_HEAD, D = 16, 64
HLOC = 8          # heads per core
CLOC = HLOC * D   # 512 local qkv channels per core
QG = 512          # query-group width
NG = T // QG      # 4 query groups
KB = 128          # key-block width
F32R = mybir.dt.float32r
F32 = mybir.dt.float32
BF16 = mybir.dt.bfloat16
EXP = mybir.ActivationFunctionType.Exp
SCALE = 1.0 / np.sqrt(D)
TRI_POOL = os.environ.get("ATTN_TRI", "pool") == "pool"
RECIP_DVE = os.environ.get("ATTN_RECIP", "dve") == "dve"

# scheduler clock model (ns)
PE_COL = 0.43          # per matmul output column, warm clock
ACT_EL = 0.833         # per exp element (per partition-lane)
ACT_FIX = 260.0        # per exp instruction overhead
SEM = 180.0            # cross-engine sem propagation
CAST_LAT = 950.0       # PSUM->SBUF cast completing after producer matmul
TRI_LAT = 650.0        # pool tri-mask mul latency after exp
NORM_LAT = 1600.0      # psy release after last AV (one DVE cast)
LEAD = 6000.0          # how far ACT may run ahead of PE before S throttles
KEEP = 1500.0          # ACT backlog above which fillers take priority over S
DMA_BW = 0.0033        # ns per byte (~300 GB/s effective)
DMA_LAT = 1900.0       # DGE issue + first-byte latency


def _build_body(nc, tc, ctx, xt, wqkt, wvt, wpt, tri, ot):
    p_wqk = ctx.enter_context(tc.tile_pool(name="wqk", bufs=8))
    p_wv = ctx.enter_context(tc.tile_pool(name="wv", bufs=8))
    p_wp = ctx.enter_context(tc.tile_pool(name="wp", bufs=4))
    p_xt = ctx.enter_context(tc.tile_pool(name="xt", bufs=16))
    p_k = ctx.enter_context(tc.tile_pool(name="ksb", bufs=4))
    p_q = ctx.enter_context(tc.tile_pool(name="qsb", bufs=8))
    p_vp = ctx.enter_context(tc.tile_pool(name="vp", bufs=16))
    p_es = ctx.enter_context(tc.tile_pool(name="es", bufs=16))
    p_ysb = ctx.enter_context(tc.tile_pool(name="ysb", bufs=5))
    p_rec = ctx.enter_context(tc.tile_pool(name="rec", bufs=1))
    p_bc = ctx.enter_context(tc.tile_pool(name="bc", bufs=8))
    p_yt = ctx.enter_context(tc.tile_pool(name="yt", bufs=16))
    p_ost = ctx.enter_context(tc.tile_pool(name="ost", bufs=4))
    p_one = ctx.enter_context(tc.tile_pool(name="one", bufs=1))
    p_drb = ctx.enter_context(tc.tile_pool(name="drb", bufs=4, space="DRAM"))
    ps_mm = ctx.enter_context(tc.tile_pool(name="psmm", bufs=2, space="PSUM"))
    ps_s = ctx.enter_context(tc.tile_pool(name="pss", bufs=2, space="PSUM"))
    ps_y = ctx.enter_context(tc.tile_pool(name="psy", bufs=1, space="PSUM"))

    # ---------------- static tensors / initial DMA batch ----------------
    dma_t = 0.0

    def dma_est(nbytes):
        nonlocal dma_t
        dma_t += nbytes * DMA_BW
        return dma_t + DMA_LAT

    wqk_sb = [None] * 8
    wv_sb = [None] * 8
    wp_sb = [None] * 4
    xt_sb = {}
    xt_ready = {}
    wqk_ready = [0.0] * 8

    def load_xt(g):
        for kc in range(8):
            t = p_xt.tile([128, QG], BF16, tag="xt", name=f"xt{g}_{kc}")
            nc.sync.dma_start(out=t, in_=xt[kc * 128:(kc + 1) * 128,
                                           g * QG:(g + 1) * QG])
            xt_sb[(g, kc)] = t
            xt_ready[(g, kc)] = dma_est(128 * QG * 2)

    # interleave x(g0) and W_qk chunks so the first qkv m-block can start
    # after one pair instead of after the full weight load; spread the issue
    # across three HWDGE queues (SP/DVE/ACT) since startup is descriptor-
    # generation limited on a single queue
    for kc in range(8):
        t = p_xt.tile([128, QG], BF16, tag="xt", name=f"xt0_{kc}")
        nc.sync.dma_start(out=t, in_=xt[kc * 128:(kc + 1) * 128, 0:QG])
        xt_sb[(0, kc)] = t
        xt_ready[(0, kc)] = dma_est(128 * QG * 2)
        w = p_wqk.tile([128, 2 * CLOC], BF16, tag="wqk", name=f"wqk{kc}")
        nc.sync.dma_start(out=w, in_=wqkt[kc * 128:(kc + 1) * 128, :])
        wqk_sb[kc] = w
        wqk_ready[kc] = dma_est(128 * 2 * CLOC * 2)
    wv_ready = [0.0] * 8
    for kc in range(8):
        w = p_wv.tile([128, CLOC], BF16, tag="wv", name=f"wv{kc}")
        nc.sync.dma_start(out=w, in_=wvt[kc * 128:(kc + 1) * 128, :])
        wv_sb[kc] = w
        wv_ready[kc] = dma_est(128 * CLOC * 2)
    tri2 = p_one.tile([128, 2, KB], BF16, tag="tri2")
    nc.sync.dma_start(out=tri2[:, 0, :], in_=tri[:, :])
    nc.sync.dma_start(out=tri2[:, 1, :], in_=tri[:, :])
    dma_est(2 * KB * KB * 2)
    ones_sb = p_one.tile([1, 64], F32R, tag="ones")
    nc.vector.memset(ones_sb.bitcast(F32), 1.0)
    # preload the Exp table during the initial DMA wait
    scr = p_one.tile([1, 64], F32, tag="scr")
    nc.scalar.activation(out=scr, in_=ones_sb.bitcast(F32), func=EXP, scale=1.0)

    def load_wp():
        for c in range(4):
            w = p_wp.tile([128, C], BF16, tag="wp", name=f"wp{c}")
            nc.sync.dma_start(out=w, in_=wpt[c * 128:(c + 1) * 128, :])
            wp_sb[c] = w
            dma_est(128 * C * 2)

    # persistent attention tensors
    k_sb = [p_k.tile([128, T], BF16, tag="ksb", name=f"ksb{c}") for c in range(4)]
    vp_sb = [None] * 16
    q_sb = {}
    yt_sb = {g: None for g in range(NG)}

    # ---------------- emission helpers ----------------
    pe_t = 0.0
    act_t = 0.0
    q_ready = {}
    v_ready = {}

    def emit_mm_block(g, m, is_v):
        """one qkv m-block: 8 accumulating matmuls + PSUM->SBUF cast"""
        nonlocal pe_t
        ps = ps_mm.tile([128, QG], F32, tag="psmm", name=f"mm{g}_{m}_{is_v}")
        for kc in range(8):
            if is_v:
                nc.tensor.matmul(ps, xt_sb[(g, kc)][:, m * 128:(m + 1) * 128],
                                 wv_sb[kc], start=kc == 0, stop=kc == 7,
                                 skip_group_check=True)
                rdy = max(xt_ready[(g, kc)], wv_ready[kc])
            else:
                nc.tensor.matmul(ps, wqk_sb[kc][:, m * 128:(m + 1) * 128],
                                 xt_sb[(g, kc)], start=kc == 0, stop=kc == 7,
                                 skip_group_check=True)
                rdy = max(xt_ready[(g, kc)], wqk_ready[kc])
            pe_t = max(pe_t + QG * PE_COL, rdy + QG * PE_COL)
        if is_v:
            vp = p_vp.tile([128, HLOC, 65], BF16, tag="vp", name=f"vp{g}_{m}")
            nc.vector.memset(vp[:, :, 64:65], 1.0)
            nc.vector.tensor_copy(out=vp[:, :, 0:64],
                                  in_=ps.rearrange("p (h d) -> p h d", d=64))
            vp_sb[g * 4 + m] = vp
            v_ready[g * 4 + m] = pe_t + CAST_LAT
        elif m >= 4:
            nc.vector.tensor_copy(
                out=k_sb[m - 4][:, g * QG:(g + 1) * QG], in_=ps)
        else:
            qt = p_q.tile([128, QG], BF16, tag="qsb", name=f"q{g}_{m}")
            nc.vector.tensor_copy(out=qt, in_=ps)
            q_sb[(g, m)] = qt
            q_ready[(g, m)] = pe_t + CAST_LAT

    def emit_S(u):
        nonlocal pe_t, act_t
        g, hp, kb = u["g"], u["hp"], u["kb"]
        c0 = u["c0"]
        vis = slice(c0, QG)
        w = QG - c0
        ps = ps_s.tile([128, 2, QG], F32, tag="pss", name=f"s{g}_{hp}_{kb}")
        for r in (0, 1):
            row = slice(64 * r, 64 * r + 64)
            nc.tensor.matmul(ps[:, r, vis],
                             k_sb[hp][row, kb * 128:(kb + 1) * 128],
                             q_sb[(g, hp)][row, vis], start=True, stop=True,
                             skip_group_check=True)
        pe_t = max(pe_t, u["rdy"]) + 2 * w * PE_COL
        es = p_es.tile([128, 2, QG], BF16, tag="es", name=f"e{g}_{hp}_{kb}")
        nc.scalar.activation(out=es[:, :, vis], in_=ps[:, :, vis],
                             func=EXP, scale=SCALE)
        act_t = max(act_t, pe_t + SEM) + 2 * w * ACT_EL + ACT_FIX
        u["es_est"] = act_t + SEM
        if kb >= 4 * g:  # diagonal block: causal tri mask (on idle Pool
            # engine; DVE for the last hp where the tri latency is critical)
            last_hp = g == NG - 1 and hp == 3
            eng = nc.vector if (last_hp or not TRI_POOL) else nc.gpsimd
            eng.tensor_mul(es[:, :, c0:c0 + 128], es[:, :, c0:c0 + 128], tri2)
            u["es_est"] += 250.0 if last_hp else TRI_LAT
        u["es"] = es
        u["exp_done"] = act_t

    def emit_AV(u, psy, k_last):
        nonlocal pe_t
        g, hp, kb = u["g"], u["hp"], u["kb"]
        vis = slice(u["c0"], QG)
        w = QG - u["c0"]
        for r in (0, 1):
            nc.tensor.matmul(psy[0:65, r, vis], vp_sb[kb][:, 2 * hp + r, :],
                             u["es"][:, r, vis], start=u["av_first"],
                             stop=k_last, skip_group_check=True)
        pe_t = max(pe_t, u["av_rdy"]) + 2 * w * PE_COL

    gden = {}
    gysb = {}
    pending_muls = []
    AOP = mybir.AluOpType

    def pop_mul():
        yt, r, ysb, bc = pending_muls.pop(0)
        nc.vector.scalar_tensor_tensor(
            out=yt[64 * r:64 * r + 64, :], in0=ysb[0:64, r, :],
            scalar=-1.0, in1=bc, op0=AOP.mult, op1=AOP.mult)

    def emit_norm(g, hp, psy, pe_bcast):
        """softmax denominators + normalize: PSUM psy -> SBUF yt (bf16).
        Non-tail groups batch 1/denominator as a Newton-seed reciprocal on
        DVE, once per group, keeping the saturated ACT engine exp-only."""
        nonlocal pe_t, act_t
        if yt_sb[g] is None:
            yt_sb[g] = [None] * 4
        ysb = p_ysb.tile([65, 2, QG], F32, tag="ysb", name=f"yb{g}_{hp}")
        if pe_bcast:
            # kernel tail: batched Ln+Exp straight off the PSUM denominator
            # rows; the broadcast matmul then reads the rec rows directly
            yt = p_yt.tile([128, QG], BF16, tag="yt", name=f"yt{g}_{hp}")
            yt_sb[g][hp] = yt
            ln2 = p_one.tile([1, 2, QG], F32, tag="ln2", name=f"ln2{g}_{hp}")
            nc.scalar.activation(out=ln2, in_=psy[64:65, :, :],
                                 func=mybir.ActivationFunctionType.Ln)
            rec2 = p_one.tile([1, 2, QG], F32R, tag="rec2", name=f"rc2{g}_{hp}")
            nc.scalar.activation(out=rec2, in_=ln2, func=EXP, scale=-1.0)
            nc.vector.tensor_copy(out=ysb, in_=psy[0:65, :, :])
            psb = ps_y.tile([128, 2, QG], F32, tag="psy", name=f"pb{g}_{hp}")
            for r in (0, 1):
                nc.tensor.matmul(psb[0:64, r, :], ones_sb, rec2[:, r, :],
                                 start=True, stop=True, skip_group_check=True)
                pe_t += QG * PE_COL
                nc.vector.tensor_mul(yt[64 * r:64 * r + 64, :],
                                     ysb[0:64, r, :], psb[0:64, r, :])
            return
        # single cast frees the PSUM psy tile fast (next hp's AVs wait on
        # it); denominator rows then come from the SBUF copy
        nc.vector.tensor_copy(out=ysb, in_=psy[0:65, :, :])
        if g not in gden:
            gden[g] = p_rec.tile([8, QG], F32, tag="gd", name=f"gd{g}")
            gysb[g] = {}
        gysb[g][hp] = ysb
        # gather both denominator rows with one small DMA (engine writes
        # must start at partition 0/32/64/96; DMA has no such restriction)
        nc.sync.dma_start(out=gden[g][2 * hp:2 * hp + 2, :],
                          in_=ysb[64:65, :, :])
        last_hp = 2 if g == NG - 1 else 3
        if hp != last_hp:
            return
        # one Newton-reciprocal chain for the whole group's 6-8 denominators
        # (sign-carried: y2 = -1/d; the final muls fold in the -1)
        d = gden[g]
        nb = p_rec.tile([8, QG], F32, tag="nb", name=f"nb{g}")
        I32 = mybir.dt.int32
        nc.vector.tensor_tensor(out=nb.bitcast(I32), in0=d.bitcast(I32),
                                in1=d.bitcast(I32), op=AOP.bitwise_not)
        y0 = p_rec.tile([8, QG], F32, tag="y0", name=f"y0{g}")
        nc.vector.tensor_scalar_mul(out=y0, in0=nb, scalar1=0.23549792)
        t1 = p_rec.tile([8, QG], F32, tag="t1", name=f"t1{g}")
        nc.vector.tensor_mul(t1, d, y0)
        y1 = p_rec.tile([8, QG], F32, tag="y1", name=f"y1{g}")
        nc.vector.scalar_tensor_tensor(out=y1, in0=t1, scalar=2.0017324,
                                       in1=y0, op0=AOP.add, op1=AOP.mult)
        t2 = p_rec.tile([8, QG], F32, tag="t2", name=f"t2{g}")
        nc.vector.tensor_mul(t2, d, y1)
        y2 = p_rec.tile([8, QG], F32, tag="y2", name=f"y2{g}")
        nc.vector.scalar_tensor_tensor(out=y2, in0=t2, scalar=2.0,
                                       in1=y1, op0=AOP.add, op1=AOP.mult)
        drec = p_drb.tile([8, QG], F32, tag="drec", name=f"dr{g}")
        nc.sync.dma_start(out=drec, in_=y2)
        for hq in range(last_hp + 1):
            yt = p_yt.tile([128, QG], BF16, tag="yt", name=f"yt{g}_{hq}")
            yt_sb[g][hq] = yt
            for r in (0, 1):
                j = 2 * hq + r
                bc = p_bc.tile([64, QG], F32, tag="bc", name=f"bc{g}_{j}")
                nc.sync.dma_start(
                    out=bc, in_=drec[j:j + 1, :].to_broadcast([64, QG]))
                # defer the mul: the scheduler spreads these through the DVE
                # stream so the 8-op burst doesn't delay qkv/proj PSUM casts
                pending_muls.append((yt, r, gysb[g][hq], bc))

    p3_tiles = {}

    def emit_proj(g, m, c_lo=0, ps=None):
        nonlocal pe_t
        if ps is None:
            ps = ps_mm.tile([128, QG], F32, tag="psmm", name=f"pj{g}_{m}")
        for c in range(c_lo, 4):
            nc.tensor.matmul(ps, wp_sb[c][:, m * 128:(m + 1) * 128],
                             yt_sb[g][c], start=c == 0, stop=c == 3,
                             skip_group_check=True)
        pe_t += (4 - c_lo) * QG * PE_COL
        ost = p_ost.tile([128, QG], BF16, tag="ost", name=f"o{g}_{m}")
        nc.vector.tensor_copy(out=ost, in_=ps)
        # final group's stores go out on the (idle-by-then) ACT queue so the
        # kernel tail doesn't wait behind the SP queue's issue backlog
        eng = nc.scalar if g == NG - 1 else nc.sync
        eng.dma_start(out=ot[m * 128:(m + 1) * 128, g * QG:(g + 1) * QG],
                      in_=ost)

    def emit_proj3_partial(m, ps=None):
        # first 3 contraction chunks of a final-group proj block; the last
        # chunk + store happen in the tail once hp3's yt lands
        nonlocal pe_t
        if ps is None:
            ps = ps_mm.tile([128, QG], F32, tag="psmm", name=f"pj3p_{m}")
        for c in range(3):
            nc.tensor.matmul(ps, wp_sb[c][:, m * 128:(m + 1) * 128],
                             yt_sb[3][c], start=c == 0, stop=False,
                             skip_group_check=True)
        pe_t += 3 * QG * PE_COL
        p3_tiles[m] = ps

    def emit_proj3_partial2(m0):
        # a pair of partial blocks sharing one (by-then idle) S-pool tile
        ps2 = ps_s.tile([128, 2, QG], F32, tag="pss", name=f"pj3q_{m0}")
        emit_proj3_partial(m0, ps2[:, 0, :])
        emit_proj3_partial(m0 + 1, ps2[:, 1, :])

    # ---------------- unit and filler lists ----------------
    # per hp, diagonal key blocks go FIRST: their tri-mask latency then
    # overlaps later units, and the hp's final AV (which releases the psy
    # chain / kernel tail) is an unmasked block
    units = []
    for g in range(NG):
        for hp in range(4):
            order = list(range(4 * g, 4 * (g + 1))) + list(range(4 * g))
            for pos, kb in enumerate(order):
                units.append({"g": g, "hp": hp, "kb": kb,
                              "c0": max(0, 128 * (kb - 4 * g)),
                              "av_first": pos == 0,
                              "av_last": pos == len(order) - 1})
    fillers = []
    for g in range(NG):
        if g > 0:
            fillers.append(("xt", g))
        if g == 1:
            fillers.append(("wp",))
        for hp in range(4):
            fillers.append(("kq", g, 4 + hp))  # k chunk
            fillers.append(("kq", g, hp))      # q chunk
        for tb in range(4):
            fillers.append(("v", g, tb))
    for g in range(NG - 1):
        for m in range(8):
            fillers.append(("proj", g, m))

    # ---------------- greedy clock-driven scheduler ----------------
    s_idx = 0
    f_idx = 0
    av_units = []          # exp-emitted units awaiting AV, lex order
    exp_done_hist = []     # S psum recycle tracking (pool depth 2)
    psy_free_est = 0.0
    yt_ready = {}
    cur_psy = None
    cur_av_key = None      # (g, hp) whose AVs are in flight

    def s_deps(u):
        gk = u["kb"] // 4
        qr = q_ready.get((u["g"], u["hp"]))
        if qr is None or (gk, u["hp"]) not in k_emitted:
            return None
        return max(qr, k_ready.get((gk, u["hp"]), 0.0))

    k_emitted = set()
    k_ready = {}

    def filler_ok(f):
        if f[0] == "proj":
            return f[1] in yt_ready
        if f[0] in ("p3a", "p3b"):
            # pure-tail fill: emit only after the last AV so the partial
            # matmuls don't push the critical-path AVs back in the PE queue
            # (p3b also takes an S-pool tile, unsafe while S units remain)
            return s_idx >= len(units) and not av_units
        return True

    def run_filler(f):
        nonlocal pe_t
        if f[0] == "xt":
            load_xt(f[1])
        elif f[0] == "wp":
            load_wp()
        elif f[0] == "kq":
            g, m = f[1], f[2]
            emit_mm_block(g, m, False)
            if m >= 4:
                k_emitted.add((g, m - 4))
                k_ready[(g, m - 4)] = pe_t + CAST_LAT
        elif f[0] == "v":
            emit_mm_block(f[1], f[2], True)
        elif f[0] == "p3a":
            emit_proj3_partial(f[1])
        elif f[0] == "p3b":
            emit_proj3_partial2(f[1])
        elif f[0] == "proj":
            pe_t = max(pe_t, yt_ready[f[1]])
            emit_proj(f[1], f[2])

    def do_av(u, forced):
        nonlocal pe_t, psy_free_est, cur_av_key, cur_psy
        key = (u["g"], u["hp"])
        if cur_av_key is None:
            if forced:
                pe_t = max(pe_t, psy_free_est)
            cur_psy = ps_y.tile([128, 2, QG], F32, tag="psy",
                                name=f"py{u['g']}_{u['hp']}")
            cur_av_key = key
        av_units.pop(0)
        u["av_rdy"] = max(u["es_est"], v_ready.get(u["kb"], 0.0))
        k_last = u["av_last"]
        emit_AV(u, cur_psy, k_last)
        if k_last:
            g, hp = key
            pe_bcast = g == NG - 1 and hp == 3
            emit_norm(g, hp, cur_psy, pe_bcast)
            psy_free_est = pe_t + NORM_LAT
            if hp == 3:
                yt_ready[g] = pe_t + NORM_LAT + 600.0
            if g == NG - 1 and hp == 2:
                fillers.append(("p3a", 0))
                fillers.append(("p3a", 1))
                fillers.append(("p3b", 2))
                fillers.append(("p3b", 4))
            cur_av_key = None
            cur_psy = None

    def try_S():
        nonlocal s_idx
        u = units[s_idx]
        rdy = s_deps(u)
        depth_ok = (len(exp_done_hist) < 2
                    or exp_done_hist[-2] <= pe_t + 250)
        if (rdy is not None and rdy <= pe_t + 250
                and act_t <= pe_t + LEAD and depth_ok
                and len(av_units) < 15):
            u["rdy"] = rdy
            emit_S(u)
            exp_done_hist.append(u["exp_done"])
            av_units.append(u)
            s_idx += 1
            return True
        return False

    while s_idx < len(units) or av_units or f_idx < len(fillers):
        if pending_muls:
            pop_mul()
        # 1) AV whose es is (estimated) ready
        if av_units:
            u = av_units[0]
            key = (u["g"], u["hp"])
            ok = (u["es_est"] <= pe_t + 60
                  and v_ready.get(u["kb"], 1e18) <= pe_t + 60)
            if ok and cur_av_key is None:
                ok = psy_free_est <= pe_t + 60
            if ok and (cur_av_key is None or cur_av_key == key):
                do_av(u, False)
                continue
        # 2) when ACT already has a healthy backlog, race qkv fillers forward
        # (kq blocks unlock the NEXT group's exp work - emitting them early
        # lets attention pull forward across window boundaries; proj blocks
        # don't enable anything, keep them in reserve for exp-bound gaps)
        act_healthy = act_t > pe_t + KEEP
        if (act_healthy and f_idx < len(fillers)
                and fillers[f_idx][0] != "proj"
                and filler_ok(fillers[f_idx])):
            run_filler(fillers[f_idx])
            f_idx += 1
            continue
        # 3) S unit if deps ready and ACT not over-backlogged
        if s_idx < len(units) and try_S():
            continue
        # 4) filler
        if f_idx < len(fillers) and filler_ok(fillers[f_idx]):
            run_filler(fillers[f_idx])
            f_idx += 1
            continue
        # 5) forced progress (stall): prefer AV, then S, then proj
        if av_units:
            u = av_units[0]
            key = (u["g"], u["hp"])
            if cur_av_key is None or cur_av_key == key:
                do_av(u, True)
                continue
        if s_idx < len(units):
            u = units[s_idx]
            rdy = s_deps(u)
            if rdy is not None:
                pe_t = max(pe_t, rdy)
                if len(exp_done_hist) >= 2:
                    pe_t = max(pe_t, exp_done_hist[-2])
                u["rdy"] = rdy
                emit_S(u)
                exp_done_hist.append(u["exp_done"])
                av_units.append(u)
                s_idx += 1
                continue
        if f_idx < len(fillers):
            f = fillers[f_idx]
            if f[0] == "proj":
                pe_t = max(pe_t, yt_ready.get(f[1], pe_t))
            run_filler(f)
            f_idx += 1
            continue
        raise RuntimeError("scheduler wedged")

    while pending_muls:
        pop_mul()
    # tail: final group's proj (finish the pre-accumulated blocks first)
    for m in range(8):
        if m in p3_tiles:
            emit_proj(NG - 1, m, c_lo=3, ps=p3_tiles[m])
        else:
            emit_proj(NG - 1, m)


def _build_nc():
    from contextlib import ExitStack
    nc = bass.Bass(trn_type="TRN2")
    xt = nc.dram_tensor("xt", [C, T], BF16, kind="ExternalInput")
    wqkt = nc.dram_tensor("wqkt", [C, 2 * CLOC], BF16, kind="ExternalInput")
    wvt = nc.dram_tensor("wvt", [C, CLOC], BF16, kind="ExternalInput")
    wpt = nc.dram_tensor("wpt", [CLOC, C], BF16, kind="ExternalInput")
    tri = nc.dram_tensor("tri", [KB, KB], BF16, kind="ExternalInput")
    ot = nc.dram_tensor("ot", [C, T], BF16, kind="ExternalOutput")
    with tile.TileContext(nc) as tc:
        with ExitStack() as ctx:
            _build_body(nc, tc, ctx, xt, wqkt, wvt, wpt, tri, ot)
    return nc


LAST_RESULTS = None
_NC_CACHE = None


def kernel(x, W_qkv, W_proj):
    global LAST_RESULTS, _NC_CACHE
    import ml_dtypes
    x = np.asarray(x, dtype=np.float32)
    W_qkv = np.asarray(W_qkv, dtype=np.float32)
    W_proj = np.asarray(W_proj, dtype=np.float32)

    if _NC_CACHE is None:
        _NC_CACHE = _build_nc()
    nc = _NC_CACHE
    _conv = lambda a: a.astype(ml_dtypes.bfloat16)
    tri = np.ascontiguousarray(np.triu(np.ones((KB, KB), np.float32)))
    in_maps = []
    for core in range(8):
        b, hg = core // 2, core % 2
        rq = slice(CLOC * hg, CLOC * hg + CLOC)
        Wq = W_qkv[0:C][rq]
        Wk = W_qkv[C:2 * C][rq]
        Wv = W_qkv[2 * C:3 * C][rq]
        in_maps.append({
            "xt": _conv(np.ascontiguousarray(x[b].T)),
            "wqkt": _conv(np.ascontiguousarray(np.concatenate([Wq, Wk], axis=0).T)),
            "wvt": _conv(np.ascontiguousarray(Wv.T)),
            "wpt": _conv(np.ascontiguousarray(W_proj[:, rq].T)),
            "tri": _conv(tri),
        })

    trace = os.environ.get("ATTN_BASS_TRACE") == "1"
    res = None
    last_exc = None
    for attempt in range(3):
        try:
            res = run_bass_kernel_spmd(nc, in_maps, core_ids=list(range(8)),
                                       trace=trace)
            break
        except Exception as e:  # transient NRT device errors happen
            last_exc = e
            import time as _time
            _time.sleep(2.0)
    if res is None:
        raise last_exc
    LAST_RESULTS = res
    out = np.empty((B, T, C), np.float32)
    for b in range(B):
        out[b] = (res.results[2 * b]["ot"].astype(np.float32)
                  + res.results[2 * b + 1]["ot"].astype(np.float32)).T
    return out


# revision 91
# speedup vs baseline: 1.0267x; 1.0139x over previous
"""Trainium2 Bass kernel for NanoAttention (B=4, T=2048, C=1024, H=16, causal).

Sharding: 8 cores = 4 batches x 2 head-groups (8 heads each).

v2: list-scheduled emission. The attention inner loop is ACT-bound (the
softmax exp on the Scalar engine costs ~2x the S+AV matmul PE time), so the
builder runs a clock-tracking greedy scheduler that interleaves qkv / proj
matmul blocks into the exp-latency gaps, keeps the PE continuously busy (which
also keeps it in the fast p-state), and starts the first matmul as soon as the
first x^T/W chunks land instead of after all weights.

Engine division of labor:
  PE     - all matmuls (qkv, S, AV, proj, tail reciprocal-broadcast)
  ACT    - only the softmax exp (the bottleneck op, nothing else)
  DVE    - PSUM->SBUF casts, softmax denominators via reciprocal_approx_fast
           (replaces the baseline's Ln+Exp ACT chain), normalize muls
  Pool   - causal tri-mask muls (SBUF-only, otherwise idle engine)
  DMA    - loads (priority-ordered), broadcast of 1/denom rows, bf16 stores
"""
import os
import sys

sys.path.insert(0, '/opt/trn_rl_repo')

import numpy as np
import orjson

import concourse.bass as bass
import concourse.mybir as mybir
import concourse.tile as tile
from concourse.bass_utils import run_bass_kernel_spmd

# ---------------------------------------------------------------------------
# Workaround for this container's walrus build: it enforces the HW limit of
# one sync-wait per instruction (two for EventSemaphore), but Tile's sem
# assignment can emit more (kernel-tail Drain waits on every DMA queue used;
# HWDGE stores can pick up two queue waits). Split the overflow onto
# preceding pure-wait EventSemaphore instructions on the same engine at
# JSON-serialization time so every compile path is covered.
# ---------------------------------------------------------------------------


def _split_multi_waits(data):
    n_split = 0
    for func in data.get("functions", []):
        for blk in func.get("blocks", []):
            insts = blk.get("instructions")
            if not insts:
                continue
            out = []
            for inst in insts:
                si = inst.get("sync_info")
                waits = (si or {}).get("on_wait") or []
                cap = 2 if inst.get("opcode") == "EventSemaphore" else 1
                if len(waits) > cap and "engine" in inst:
                    extra = waits[:-cap]
                    si["on_wait"] = waits[-cap:]
                    for i in range(0, len(extra), 2):
                        n_split += 1
                        out.append({
                            "debug": inst.get("debug"),
                            "engine": inst["engine"],
                            "ins": [],
                            "outs": [],
                            "name": f"{inst['name']}_wsplit{n_split}",
                            "opcode": "EventSemaphore",
                            "sync_info": {"on_wait": extra[i:i + 2],
                                          "on_update": []},
                        })
                out.append(inst)
            blk["instructions"] = out
    return data


_orig_to_json_bytes = bass.Bass.to_json_bytes


def _patched_to_json_bytes(self):
    return orjson.dumps(_split_multi_waits(orjson.loads(_orig_to_json_bytes(self))))


bass.Bass.to_json_bytes = _patched_to_json_bytes

# ---------------------------------------------------------------------------

B, T, C = 4, 2048, 1024
N_HEAD, D = 16, 64
HLOC = 8          # heads per core
CLOC = HLOC * D   # 512 local qkv channels per core
QG = 512          # query-group width
NG = T // QG      # 4 query groups
KB = 128          # key-block width
F32R = mybir.dt.float32r
F32 = mybir.dt.float32
BF16 = mybir.dt.bfloat16
EXP = mybir.ActivationFunctionType.Exp
SCALE = 1.0 / np.sqrt(D)
TRI_POOL = os.environ.get("ATTN_TRI", "pool") == "pool"
RECIP_DVE = os.environ.get("ATTN_RECIP", "dve") == "dve"

# scheduler clock model (ns)
PE_COL = 0.43          # per matmul output column, warm clock
ACT_EL = 0.833         # per exp element (per partition-lane)
ACT_FIX = 260.0        # per exp instruction overhead
SEM = 180.0            # cross-engine sem propagation
CAST_LAT = 950.0       # PSUM->SBUF cast completing after producer matmul
TRI_LAT = 650.0        # pool tri-mask mul latency after exp
NORM_LAT = 1600.0      # psy release after last AV (one DVE cast)
LEAD = 6000.0          # how far ACT may run ahead of PE before S throttles
KEEP = 1500.0          # ACT backlog above which fillers take priority over S
DMA_BW = 0.0033        # ns per byte (~300 GB/s effective)
DMA_LAT = 1900.0       # DGE issue + first-byte latency


def _build_body(nc, tc, ctx, xt, wqkt, wvt, wpt, tri, ot):
    p_wqk = ctx.enter_context(tc.tile_pool(name="wqk", bufs=8))
    p_wv = ctx.enter_context(tc.tile_pool(name="wv", bufs=8))
    p_wp = ctx.enter_context(tc.tile_pool(name="wp", bufs=4))
    p_xt = ctx.enter_context(tc.tile_pool(name="xt", bufs=16))
    p_k = ctx.enter_context(tc.tile_pool(name="ksb", bufs=4))
    p_q = ctx.enter_context(tc.tile_pool(name="qsb", bufs=8))
    p_vp = ctx.enter_context(tc.tile_pool(name="vp", bufs=16))
    p_es = ctx.enter_context(tc.tile_pool(name="es", bufs=16))
    p_ysb = ctx.enter_context(tc.tile_pool(name="ysb", bufs=5))
    p_rec = ctx.enter_context(tc.tile_pool(name="rec", bufs=1))
    p_bc = ctx.enter_context(tc.tile_pool(name="bc", bufs=8))
    p_yt = ctx.enter_context(tc.tile_pool(name="yt", bufs=16))
    p_ost = ctx.enter_context(tc.tile_pool(name="ost", bufs=4))
    p_one = ctx.enter_context(tc.tile_pool(name="one", bufs=1))
    p_drb = ctx.enter_context(tc.tile_pool(name="drb", bufs=4, space="DRAM"))
    ps_mm = ctx.enter_context(tc.tile_pool(name="psmm", bufs=2, space="PSUM"))
    ps_s = ctx.enter_context(tc.tile_pool(name="pss", bufs=2, space="PSUM"))
    ps_y = ctx.enter_context(tc.tile_pool(name="psy", bufs=1, space="PSUM"))

    # ---------------- static tensors / initial DMA batch ----------------
    dma_t = 0.0

    def dma_est(nbytes):
        nonlocal dma_t
        dma_t += nbytes * DMA_BW
        return dma_t + DMA_LAT

    wqk_sb = [None] * 8
    wv_sb = [None] * 8
    wp_sb = [None] * 4
    xt_sb = {}
    xt_ready = {}
    wqk_ready = [0.0] * 8

    def load_xt(g):
        for kc in range(8):
            t = p_xt.tile([128, QG], BF16, tag="xt", name=f"xt{g}_{kc}")
            nc.sync.dma_start(out=t, in_=xt[kc * 128:(kc + 1) * 128,
                                           g * QG:(g + 1) * QG])
            xt_sb[(g, kc)] = t
            xt_ready[(g, kc)] = dma_est(128 * QG * 2)

    # interleave x(g0) and W_qk chunks so the first qkv m-block can start
    # after one pair instead of after the full weight load; spread the issue
    # across three HWDGE queues (SP/DVE/ACT) since startup is descriptor-
    # generation limited on a single queue
    for kc in range(8):
        t = p_xt.tile([128, QG], BF16, tag="xt", name=f"xt0_{kc}")
        nc.sync.dma_start(out=t, in_=xt[kc * 128:(kc + 1) * 128, 0:QG])
        xt_sb[(0, kc)] = t
        xt_ready[(0, kc)] = dma_est(128 * QG * 2)
        w = p_wqk.tile([128, 2 * CLOC], BF16, tag="wqk", name=f"wqk{kc}")
        nc.sync.dma_start(out=w, in_=wqkt[kc * 128:(kc + 1) * 128, :])
        wqk_sb[kc] = w
        wqk_ready[kc] = dma_est(128 * 2 * CLOC * 2)
    wv_ready = [0.0] * 8
    for kc in range(8):
        w = p_wv.tile([128, CLOC], BF16, tag="wv", name=f"wv{kc}")
        nc.sync.dma_start(out=w, in_=wvt[kc * 128:(kc + 1) * 128, :])
        wv_sb[kc] = w
        wv_ready[kc] = dma_est(128 * CLOC * 2)
    tri2 = p_one.tile([128, 2, KB], BF16, tag="tri2")
    nc.sync.dma_start(out=tri2[:, 0, :], in_=tri[:, :])
    nc.sync.dma_start(out=tri2[:, 1, :], in_=tri[:, :])
    dma_est(2 * KB * KB * 2)
    ones_sb = p_one.tile([1, 64], F32R, tag="ones")
    nc.vector.memset(ones_sb.bitcast(F32), 1.0)
    # preload the Exp table during the initial DMA wait
    scr = p_one.tile([1, 64], F32, tag="scr")
    nc.scalar.activation(out=scr, in_=ones_sb.bitcast(F32), func=EXP, scale=1.0)

    def load_wp():
        for c in range(4):
            w = p_wp.tile([128, C], BF16, tag="wp", name=f"wp{c}")
            nc.sync.dma_start(out=w, in_=wpt[c * 128:(c + 1) * 128, :])
            wp_sb[c] = w
            dma_est(128 * C * 2)

    # persistent attention tensors
    k_sb = [p_k.tile([128, T], BF16, tag="ksb", name=f"ksb{c}") for c in range(4)]
    vp_sb = [None] * 16
    q_sb = {}
    yt_sb = {g: None for g in range(NG)}

    # ---------------- emission helpers ----------------
    pe_t = 0.0
    act_t = 0.0
    q_ready = {}
    v_ready = {}

    def emit_mm_block(g, m, is_v):
        """one qkv m-block: 8 accumulating matmuls + PSUM->SBUF cast"""
        nonlocal pe_t
        ps = ps_mm.tile([128, QG], F32, tag="psmm", name=f"mm{g}_{m}_{is_v}")
        for kc in range(8):
            if is_v:
                nc.tensor.matmul(ps, xt_sb[(g, kc)][:, m * 128:(m + 1) * 128],
                                 wv_sb[kc], start=kc == 0, stop=kc == 7,
                                 skip_group_check=True)
                rdy = max(xt_ready[(g, kc)], wv_ready[kc])
            else:
                nc.tensor.matmul(ps, wqk_sb[kc][:, m * 128:(m + 1) * 128],
                                 xt_sb[(g, kc)], start=kc == 0, stop=kc == 7,
                                 skip_group_check=True)
                rdy = max(xt_ready[(g, kc)], wqk_ready[kc])
            pe_t = max(pe_t + QG * PE_COL, rdy + QG * PE_COL)
        if is_v:
            vp = p_vp.tile([128, HLOC, 65], BF16, tag="vp", name=f"vp{g}_{m}")
            nc.vector.memset(vp[:, :, 64:65], 1.0)
            nc.vector.tensor_copy(out=vp[:, :, 0:64],
                                  in_=ps.rearrange("p (h d) -> p h d", d=64))
            vp_sb[g * 4 + m] = vp
            v_ready[g * 4 + m] = pe_t + CAST_LAT
        elif m >= 4:
            nc.vector.tensor_copy(
                out=k_sb[m - 4][:, g * QG:(g + 1) * QG], in_=ps)
        else:
            qt = p_q.tile([128, QG], BF16, tag="qsb", name=f"q{g}_{m}")
            nc.vector.tensor_copy(out=qt, in_=ps)
            q_sb[(g, m)] = qt
            q_ready[(g, m)] = pe_t + CAST_LAT

    def emit_S(u):
        nonlocal pe_t, act_t
        g, hp, kb = u["g"], u["hp"], u["kb"]
        c0 = u["c0"]
        vis = slice(c0, QG)
        w = QG - c0
        ps = ps_s.tile([128, 2, QG], F32, tag="pss", name=f"s{g}_{hp}_{kb}")
        for r in (0, 1):
            row = slice(64 * r, 64 * r + 64)
            nc.tensor.matmul(ps[:, r, vis],
                             k_sb[hp][row, kb * 128:(kb + 1) * 128],
                             q_sb[(g, hp)][row, vis], start=True, stop=True,
                             skip_group_check=True)
        pe_t = max(pe_t, u["rdy"]) + 2 * w * PE_COL
        es = p_es.tile([128, 2, QG], BF16, tag="es", name=f"e{g}_{hp}_{kb}")
        nc.scalar.activation(out=es[:, :, vis], in_=ps[:, :, vis],
                             func=EXP, scale=SCALE)
        act_t = max(act_t, pe_t + SEM) + 2 * w * ACT_EL + ACT_FIX
        u["es_est"] = act_t + SEM
        if kb >= 4 * g:  # diagonal block: causal tri mask (on idle Pool
            # engine; DVE for the last hp where the tri latency is critical)
            last_hp = g == NG - 1 and hp == 3
            eng = nc.vector if (last_hp or not TRI_POOL) else nc.gpsimd
            eng.tensor_mul(es[:, :, c0:c0 + 128], es[:, :, c0:c0 + 128], tri2)
            u["es_est"] += 250.0 if last_hp else TRI_LAT
        u["es"] = es
        u["exp_done"] = act_t

    def emit_AV(u, psy, k_last):
        nonlocal pe_t
        g, hp, kb = u["g"], u["hp"], u["kb"]
        vis = slice(u["c0"], QG)
        w = QG - u["c0"]
        for r in (0, 1):
            nc.tensor.matmul(psy[0:65, r, vis], vp_sb[kb][:, 2 * hp + r, :],
                             u["es"][:, r, vis], start=u["av_first"],
                             stop=k_last, skip_group_check=True)
        pe_t = max(pe_t, u["av_rdy"]) + 2 * w * PE_COL

    gden = {}
    gysb = {}
    pending_muls = []
    AOP = mybir.AluOpType

    def pop_mul():
        yt, r, ysb, bc = pending_muls.pop(0)
        nc.vector.scalar_tensor_tensor(
            out=yt[64 * r:64 * r + 64, :], in0=ysb[0:64, r, :],
            scalar=-1.0, in1=bc, op0=AOP.mult, op1=AOP.mult)

    def emit_norm(g, hp, psy, pe_bcast):
        """softmax denominators + normalize: PSUM psy -> SBUF yt (bf16).
        Non-tail groups batch 1/denominator as a Newton-seed reciprocal on
        DVE, once per group, keeping the saturated ACT engine exp-only."""
        nonlocal pe_t, act_t
        if yt_sb[g] is None:
            yt_sb[g] = [None] * 4
        ysb = p_ysb.tile([65, 2, QG], F32, tag="ysb", name=f"yb{g}_{hp}")
        if pe_bcast:
            # kernel tail: batched Ln+Exp straight off the PSUM denominator
            # rows; the broadcast matmul then reads the rec rows directly
            yt = p_yt.tile([128, QG], BF16, tag="yt", name=f"yt{g}_{hp}")
            yt_sb[g][hp] = yt
            ln2 = p_one.tile([1, 2, QG], F32, tag="ln2", name=f"ln2{g}_{hp}")
            nc.scalar.activation(out=ln2, in_=psy[64:65, :, :],
                                 func=mybir.ActivationFunctionType.Ln)
            rec2 = p_one.tile([1, 2, QG], F32R, tag="rec2", name=f"rc2{g}_{hp}")
            nc.scalar.activation(out=rec2, in_=ln2, func=EXP, scale=-1.0)
            nc.vector.tensor_copy(out=ysb, in_=psy[0:65, :, :])
            psb = ps_y.tile([128, 2, QG], F32, tag="psy", name=f"pb{g}_{hp}")
            for r in (0, 1):
                nc.tensor.matmul(psb[0:64, r, :], ones_sb, rec2[:, r, :],
                                 start=True, stop=True, skip_group_check=True)
                pe_t += QG * PE_COL
                nc.vector.tensor_mul(yt[64 * r:64 * r + 64, :],
                                     ysb[0:64, r, :], psb[0:64, r, :])
            return
        # single cast frees the PSUM psy tile fast (next hp's AVs wait on
        # it); denominator rows then come from the SBUF copy
        nc.vector.tensor_copy(out=ysb, in_=psy[0:65, :, :])
        if g not in gden:
            gden[g] = p_rec.tile([8, QG], F32, tag="gd", name=f"gd{g}")
            gysb[g] = {}
        gysb[g][hp] = ysb
        # gather both denominator rows with one small DMA (engine writes
        # must start at partition 0/32/64/96; DMA has no such restriction)
        nc.sync.dma_start(out=gden[g][2 * hp:2 * hp + 2, :],
                          in_=ysb[64:65, :, :])
        last_hp = 2 if g == NG - 1 else 3
        if hp != last_hp:
            return
        # one Newton-reciprocal chain for the whole group's 6-8 denominators
        # (sign-carried: y2 = -1/d; the final muls fold in the -1)
        d = gden[g]
        nb = p_rec.tile([8, QG], F32, tag="nb", name=f"nb{g}")
        I32 = mybir.dt.int32
        nc.vector.tensor_tensor(out=nb.bitcast(I32), in0=d.bitcast(I32),
                                in1=d.bitcast(I32), op=AOP.bitwise_not)
        y0 = p_rec.tile([8, QG], F32, tag="y0", name=f"y0{g}")
        nc.vector.tensor_scalar_mul(out=y0, in0=nb, scalar1=0.23549792)
        t1 = p_rec.tile([8, QG], F32, tag="t1", name=f"t1{g}")
        nc.vector.tensor_mul(t1, d, y0)
        y1 = p_rec.tile([8, QG], F32, tag="y1", name=f"y1{g}")
        nc.vector.scalar_tensor_tensor(out=y1, in0=t1, scalar=2.0017324,
                                       in1=y0, op0=AOP.add, op1=AOP.mult)
        t2 = p_rec.tile([8, QG], F32, tag="t2", name=f"t2{g}")
        nc.vector.tensor_mul(t2, d, y1)
        y2 = p_rec.tile([8, QG], F32, tag="y2", name=f"y2{g}")
        nc.vector.scalar_tensor_tensor(out=y2, in0=t2, scalar=2.0,
                                       in1=y1, op0=AOP.add, op1=AOP.mult)
        drec = p_drb.tile([8, QG], F32, tag="drec", name=f"dr{g}")
        nc.sync.dma_start(out=drec, in_=y2)
        for hq in range(last_hp + 1):
            yt = p_yt.tile([128, QG], BF16, tag="yt", name=f"yt{g}_{hq}")
            yt_sb[g][hq] = yt
            for r in (0, 1):
                j = 2 * hq + r
                bc = p_bc.tile([64, QG], F32, tag="bc", name=f"bc{g}_{j}")
                nc.sync.dma_start(
                    out=bc, in_=drec[j:j + 1, :].to_broadcast([64, QG]))
                # defer the mul: the scheduler spreads these through the DVE
                # stream so the 8-op burst doesn't delay qkv/proj PSUM casts
                pending_muls.append((yt, r, gysb[g][hq], bc))

    p3_tiles = {}

    def emit_proj(g, m, c_lo=0, ps=None):
        nonlocal pe_t
        if ps is None:
            ps = ps_mm.tile([128, QG], F32, tag="psmm", name=f"pj{g}_{m}")
        for c in range(c_lo, 4):
            nc.tensor.matmul(ps, wp_sb[c][:, m * 128:(m + 1) * 128],
                             yt_sb[g][c], start=c == 0, stop=c == 3,
                             skip_group_check=True)
        pe_t += (4 - c_lo) * QG * PE_COL
        ost = p_ost.tile([128, QG], BF16, tag="ost", name=f"o{g}_{m}")
        nc.vector.tensor_copy(out=ost, in_=ps)
        # final group's stores go out on the (idle-by-then) ACT queue so the
        # kernel tail doesn't wait behind the SP queue's issue backlog
        eng = nc.scalar if g == NG - 1 else nc.sync
        eng.dma_start(out=ot[m * 128:(m + 1) * 128, g * QG:(g + 1) * QG],
                      in_=ost)

    def emit_proj3_partial(m, ps=None):
        # first 3 contraction chunks of a final-group proj block; the last
        # chunk + store happen in the tail once hp3's yt lands
        nonlocal pe_t
        if ps is None:
            ps = ps_mm.tile([128, QG], F32, tag="psmm", name=f"pj3p_{m}")
        for c in range(3):
            nc.tensor.matmul(ps, wp_sb[c][:, m * 128:(m + 1) * 128],
                             yt_sb[3][c], start=c == 0, stop=False,
                             skip_group_check=True)
        pe_t += 3 * QG * PE_COL
        p3_tiles[m] = ps

    def emit_proj3_partial2(m0):
        # a pair of partial blocks sharing one (by-then idle) S-pool tile
        ps2 = ps_s.tile([128, 2, QG], F32, tag="pss", name=f"pj3q_{m0}")
        emit_proj3_partial(m0, ps2[:, 0, :])
        emit_proj3_partial(m0 + 1, ps2[:, 1, :])

    # ---------------- unit and filler lists ----------------
    # per hp, diagonal key blocks go FIRST: their tri-mask latency then
    # overlaps later units, and the hp's final AV (which releases the psy
    # chain / kernel tail) is an unmasked block
    units = []
    for g in range(NG):
        for hp in range(4):
            order = list(range(4 * g, 4 * (g + 1))) + list(range(4 * g))
            for pos, kb in enumerate(order):
                units.append({"g": g, "hp": hp, "kb": kb,
                              "c0": max(0, 128 * (kb - 4 * g)),
                              "av_first": pos == 0,
                              "av_last": pos == len(order) - 1})
    fillers = []
    for g in range(NG):
        if g > 0:
            fillers.append(("xt", g))
        if g == 1:
            fillers.append(("wp",))
        for hp in range(4):
            fillers.append(("kq", g, 4 + hp))  # k chunk
            fillers.append(("kq", g, hp))      # q chunk
        for tb in range(4):
            fillers.append(("v", g, tb))
    for g in range(NG - 1):
        for m in range(8):
            fillers.append(("proj", g, m))

    # ---------------- greedy clock-driven scheduler ----------------
    s_idx = 0
    f_idx = 0
    av_units = []          # exp-emitted units awaiting AV, lex order
    exp_done_hist = []     # S psum recycle tracking (pool depth 2)
    psy_free_est = 0.0
    yt_ready = {}
    cur_psy = None
    cur_av_key = None      # (g, hp) whose AVs are in flight

    def s_deps(u):
        gk = u["kb"] // 4
        qr = q_ready.get((u["g"], u["hp"]))
        if qr is None or (gk, u["hp"]) not in k_emitted:
            return None
        return max(qr, k_ready.get((gk, u["hp"]), 0.0))

    k_emitted = set()
    k_ready = {}

    def filler_ok(f):
        if f[0] == "proj":
            return f[1] in yt_ready
        if f[0] in ("p3a", "p3b"):
            # pure-tail fill: emit only after the last AV so the partial
            # matmuls don't push the critical-path AVs back in the PE queue
            # (p3b also takes an S-pool tile, unsafe while S units remain)
            return s_idx >= len(units) and not av_units
        return True

    def run_filler(f):
        nonlocal pe_t
        if f[0] == "xt":
            load_xt(f[1])
        elif f[0] == "wp":
            load_wp()
        elif f[0] == "kq":
            g, m = f[1], f[2]
            emit_mm_block(g, m, False)
            if m >= 4:
                k_emitted.add((g, m - 4))
                k_ready[(g, m - 4)] = pe_t + CAST_LAT
        elif f[0] == "v":
            emit_mm_block(f[1], f[2], True)
        elif f[0] == "p3a":
            emit_proj3_partial(f[1])
        elif f[0] == "p3b":
            emit_proj3_partial2(f[1])
        elif f[0] == "proj":
            pe_t = max(pe_t, yt_ready[f[1]])
            emit_proj(f[1], f[2])

    def do_av(u, forced):
        nonlocal pe_t, psy_free_est, cur_av_key, cur_psy
        key = (u["g"], u["hp"])
        if cur_av_key is None:
            if forced:
                pe_t = max(pe_t, psy_free_est)
            cur_psy = ps_y.tile([128, 2, QG], F32, tag="psy",
                                name=f"py{u['g']}_{u['hp']}")
            cur_av_key = key
        av_units.pop(0)
        u["av_rdy"] = max(u["es_est"], v_ready.get(u["kb"], 0.0))
        k_last = u["av_last"]
        emit_AV(u, cur_psy, k_last)
        if k_last:
            g, hp = key
            pe_bcast = g == NG - 1 and hp == 3
            emit_norm(g, hp, cur_psy, pe_bcast)
            psy_free_est = pe_t + NORM_LAT
            if hp == 3:
                yt_ready[g] = pe_t + NORM_LAT + 600.0
            if g == NG - 1 and hp == 2:
                fillers.append(("p3a", 0))
                fillers.append(("p3a", 1))
                fillers.append(("p3b", 2))
                fillers.append(("p3b", 4))
            cur_av_key = None
            cur_psy = None

    def try_S():
        nonlocal s_idx
        u = units[s_idx]
        rdy = s_deps(u)
        depth_ok = (len(exp_done_hist) < 2
                    or exp_done_hist[-2] <= pe_t + 250)
        if (rdy is not None and rdy <= pe_t + 250
                and act_t <= pe_t + LEAD and depth_ok
                and len(av_units) < 15):
            u["rdy"] = rdy
            emit_S(u)
            exp_done_hist.append(u["exp_done"])
            av_units.append(u)
            s_idx += 1
            return True
        return False

    while s_idx < len(units) or av_units or f_idx < len(fillers):
        if pending_muls:
            pop_mul()
        # 1) AV whose es is (estimated) ready
        if av_units:
            u = av_units[0]
            key = (u["g"], u["hp"])
            ok = (u["es_est"] <= pe_t + 200
                  and v_ready.get(u["kb"], 1e18) <= pe_t + 200)
            if ok and cur_av_key is None:
                ok = psy_free_est <= pe_t + 60
            if ok and (cur_av_key is None or cur_av_key == key):
                do_av(u, False)
                continue
        # 2) when ACT already has a healthy backlog, race qkv fillers forward
        # (kq blocks unlock the NEXT group's exp work - emitting them early
        # lets attention pull forward across window boundaries; proj blocks
        # don't enable anything, keep them in reserve for exp-bound gaps)
        act_healthy = act_t > pe_t + KEEP
        if (act_healthy and f_idx < len(fillers)
                and fillers[f_idx][0] != "proj"
                and filler_ok(fillers[f_idx])):
            run_filler(fillers[f_idx])
            f_idx += 1
            continue
        # 3) S unit if deps ready and ACT not over-backlogged
        if s_idx < len(units) and try_S():
            continue
        # 4) filler
        if f_idx < len(fillers) and filler_ok(fillers[f_idx]):
            run_filler(fillers[f_idx])
            f_idx += 1
            continue
        # 5) forced progress (stall): prefer AV, then S, then proj
        if av_units:
            u = av_units[0]
            key = (u["g"], u["hp"])
            if cur_av_key is None or cur_av_key == key:
                do_av(u, True)
                continue
        if s_idx < len(units):
            u = units[s_idx]
            rdy = s_deps(u)
            if rdy is not None:
                pe_t = max(pe_t, rdy)
                if len(exp_done_hist) >= 2:
                    pe_t = max(pe_t, exp_done_hist[-2])
                u["rdy"] = rdy
                emit_S(u)
                exp_done_hist.append(u["exp_done"])
                av_units.append(u)
                s_idx += 1
                continue
        if f_idx < len(fillers):
            f = fillers[f_idx]
            if f[0] == "proj":
                pe_t = max(pe_t, yt_ready.get(f[1], pe_t))
            run_filler(f)
            f_idx += 1
            continue
        raise RuntimeError("scheduler wedged")

    while pending_muls:
        pop_mul()
    # tail: final group's proj (finish the pre-accumulated blocks first)
    for m in range(8):
        if m in p3_tiles:
            emit_proj(NG - 1, m, c_lo=3, ps=p3_tiles[m])
        else:
            emit_proj(NG - 1, m)


def _build_nc():
    from contextlib import ExitStack
    nc = bass.Bass(trn_type="TRN2")
    xt = nc.dram_tensor("xt", [C, T], BF16, kind="ExternalInput")
    wqkt = nc.dram_tensor("wqkt", [C, 2 * CLOC], BF16, kind="ExternalInput")
    wvt = nc.dram_tensor("wvt", [C, CLOC], BF16, kind="ExternalInput")
    wpt = nc.dram_tensor("wpt", [CLOC, C], BF16, kind="ExternalInput")
    tri = nc.dram_tensor("tri", [KB, KB], BF16, kind="ExternalInput")
    ot = nc.dram_tensor("ot", [C, T], BF16, kind="ExternalOutput")
    with tile.TileContext(nc) as tc:
        with ExitStack() as ctx:
            _build_body(nc, tc, ctx, xt, wqkt, wvt, wpt, tri, ot)
    return nc


LAST_RESULTS = None
_NC_CACHE = None


def kernel(x, W_qkv, W_proj):
    global LAST_RESULTS, _NC_CACHE
    import ml_dtypes
    x = np.asarray(x, dtype=np.float32)
    W_qkv = np.asarray(W_qkv, dtype=np.float32)
    W_proj = np.asarray(W_proj, dtype=np.float32)

    if _NC_CACHE is None:
        _NC_CACHE = _build_nc()
    nc = _NC_CACHE
    _conv = lambda a: a.astype(ml_dtypes.bfloat16)
    tri = np.ascontiguousarray(np.triu(np.ones((KB, KB), np.float32)))
    in_maps = []
    for core in range(8):
        b, hg = core // 2, core % 2
        rq = slice(CLOC * hg, CLOC * hg + CLOC)
        Wq = W_qkv[0:C][rq]
        Wk = W_qkv[C:2 * C][rq]
        Wv = W_qkv[2 * C:3 * C][rq]
        in_maps.append({
            "xt": _conv(np.ascontiguousarray(x[b].T)),
            "wqkt": _conv(np.ascontiguousarray(np.concatenate([Wq, Wk], axis=0).T)),
            "wvt": _conv(np.ascontiguousarray(Wv.T)),
            "wpt": _conv(np.ascontiguousarray(W_proj[:, rq].T)),
            "tri": _conv(tri),
        })

    trace = os.environ.get("ATTN_BASS_TRACE") == "1"
    res = None
    last_exc = None
    for attempt in range(3):
        try:
            res = run_bass_kernel_spmd(nc, in_maps, core_ids=list(range(8)),
                                       trace=trace)
            break
        except Exception as e:  # transient NRT device errors happen
            last_exc = e
            import time as _time
            _time.sleep(2.0)
    if res is None:
        raise last_exc
    LAST_RESULTS = res
    out = np.empty((B, T, C), np.float32)
    for b in range(B):
        out[b] = (res.results[2 * b]["ot"].astype(np.float32)
                  + res.results[2 * b + 1]["ot"].astype(np.float32)).T
    return out


# revision 93
# speedup vs baseline: 1.0287x; 1.0020x over previous
"""Trainium2 Bass kernel for NanoAttention (B=4, T=2048, C=1024, H=16, causal).

Sharding: 8 cores = 4 batches x 2 head-groups (8 heads each). Each core:
column-parallel qkv (q,k transposed layout, v natural with an appended ones
column that folds the softmax denominator into the AV matmul), causal
attention over 8 heads, row-parallel proj producing a bf16 [C, T] partial
that the host sums per batch pair.

The builder is a clock-tracking greedy list scheduler. The attention inner
loop is ACT-bound (the exp costs ~2x the S+AV matmul PE time), so attention
units, qkv/proj filler blocks, and normalization chains are interleaved at
emission time to keep both the PE and ACT engines continuously fed (which
also keeps the PE in its fast p-state). Attention for later query groups is
pulled forward into earlier, PE-bound windows as soon as its q/k are ready.

Engine division of labor:
  PE     - all matmuls; tail 1/denom row-broadcast via ones-matmul
  ACT    - softmax exp (the co-bottleneck: kept exp-only) + tail Ln/Exp
  DVE    - PSUM->SBUF casts, group-batched Newton reciprocal for the
           softmax denominators, normalize muls (spread between casts)
  Pool   - causal tri-mask muls (SBUF-only work for an otherwise idle engine)
  DMA    - priority-ordered loads, 1/denom row broadcasts, bf16 stores
           (final group on the ACT queue to dodge the SP issue backlog)

The kernel tail pre-accumulates the final proj blocks' first 3 contraction
chunks during the last head-pair's softmax drip, so only one matmul per
block remains after the last normalization.
"""
import os
import sys

sys.path.insert(0, '/opt/trn_rl_repo')

import numpy as np
import orjson

import concourse.bass as bass
import concourse.mybir as mybir
import concourse.tile as tile
from concourse.bass_utils import run_bass_kernel_spmd

# ---------------------------------------------------------------------------
# Workaround for this container's walrus build: it enforces the HW limit of
# one sync-wait per instruction (two for EventSemaphore), but Tile's sem
# assignment can emit more (kernel-tail Drain waits on every DMA queue used;
# HWDGE stores can pick up two queue waits). Split the overflow onto
# preceding pure-wait EventSemaphore instructions on the same engine at
# JSON-serialization time so every compile path is covered.
# ---------------------------------------------------------------------------


def _split_multi_waits(data):
    n_split = 0
    for func in data.get("functions", []):
        for blk in func.get("blocks", []):
            insts = blk.get("instructions")
            if not insts:
                continue
            out = []
            for inst in insts:
                si = inst.get("sync_info")
                waits = (si or {}).get("on_wait") or []
                cap = 2 if inst.get("opcode") == "EventSemaphore" else 1
                if len(waits) > cap and "engine" in inst:
                    extra = waits[:-cap]
                    si["on_wait"] = waits[-cap:]
                    for i in range(0, len(extra), 2):
                        n_split += 1
                        out.append({
                            "debug": inst.get("debug"),
                            "engine": inst["engine"],
                            "ins": [],
                            "outs": [],
                            "name": f"{inst['name']}_wsplit{n_split}",
                            "opcode": "EventSemaphore",
                            "sync_info": {"on_wait": extra[i:i + 2],
                                          "on_update": []},
                        })
                out.append(inst)
            blk["instructions"] = out
    return data


_orig_to_json_bytes = bass.Bass.to_json_bytes


def _patched_to_json_bytes(self):
    return orjson.dumps(_split_multi_waits(orjson.loads(_orig_to_json_bytes(self))))


bass.Bass.to_json_bytes = _patched_to_json_bytes

# ---------------------------------------------------------------------------

B, T, C = 4, 2048, 1024
N_HEAD, D = 16, 64
HLOC = 8          # heads per core
CLOC = HLOC * D   # 512 local qkv channels per core
QG = 512          # query-group width
NG = T // QG      # 4 query groups
KB = 128          # key-block width
F32R = mybir.dt.float32r
F32 = mybir.dt.float32
BF16 = mybir.dt.bfloat16
EXP = mybir.ActivationFunctionType.Exp
SCALE = 1.0 / np.sqrt(D)
TRI_POOL = os.environ.get("ATTN_TRI", "pool") == "pool"

# scheduler clock model (ns)
PE_COL = 0.43          # per matmul output column, warm clock
ACT_EL = 0.833         # per exp element (per partition-lane)
ACT_FIX = 260.0        # per exp instruction overhead
SEM = 180.0            # cross-engine sem propagation
CAST_LAT = 950.0       # PSUM->SBUF cast completing after producer matmul
TRI_LAT = 650.0        # pool tri-mask mul latency after exp
NORM_LAT = 1600.0      # psy release after last AV (one DVE cast)
LEAD = 6000.0          # how far ACT may run ahead of PE before S throttles
KEEP = 1500.0          # ACT backlog above which fillers take priority over S
DMA_BW = 0.0033        # ns per byte (~300 GB/s effective)
DMA_LAT = 1900.0       # DGE issue + first-byte latency


def _build_body(nc, tc, ctx, xt, wqkt, wvt, wpt, tri, ot):
    p_wqk = ctx.enter_context(tc.tile_pool(name="wqk", bufs=8))
    p_wv = ctx.enter_context(tc.tile_pool(name="wv", bufs=8))
    p_wp = ctx.enter_context(tc.tile_pool(name="wp", bufs=4))
    p_xt = ctx.enter_context(tc.tile_pool(name="xt", bufs=16))
    p_k = ctx.enter_context(tc.tile_pool(name="ksb", bufs=4))
    p_q = ctx.enter_context(tc.tile_pool(name="qsb", bufs=8))
    p_vp = ctx.enter_context(tc.tile_pool(name="vp", bufs=16))
    p_es = ctx.enter_context(tc.tile_pool(name="es", bufs=16))
    p_ysb = ctx.enter_context(tc.tile_pool(name="ysb", bufs=5))
    p_rec = ctx.enter_context(tc.tile_pool(name="rec", bufs=1))
    p_bc = ctx.enter_context(tc.tile_pool(name="bc", bufs=8))
    p_yt = ctx.enter_context(tc.tile_pool(name="yt", bufs=16))
    p_ost = ctx.enter_context(tc.tile_pool(name="ost", bufs=4))
    p_one = ctx.enter_context(tc.tile_pool(name="one", bufs=1))
    p_drb = ctx.enter_context(tc.tile_pool(name="drb", bufs=4, space="DRAM"))
    ps_mm = ctx.enter_context(tc.tile_pool(name="psmm", bufs=2, space="PSUM"))
    ps_s = ctx.enter_context(tc.tile_pool(name="pss", bufs=2, space="PSUM"))
    ps_y = ctx.enter_context(tc.tile_pool(name="psy", bufs=1, space="PSUM"))

    # ---------------- static tensors / initial DMA batch ----------------
    dma_t = 0.0

    def dma_est(nbytes):
        nonlocal dma_t
        dma_t += nbytes * DMA_BW
        return dma_t + DMA_LAT

    wqk_sb = [None] * 8
    wv_sb = [None] * 8
    wp_sb = [None] * 4
    xt_sb = {}
    xt_ready = {}
    wqk_ready = [0.0] * 8

    def load_xt(g):
        for kc in range(8):
            t = p_xt.tile([128, QG], BF16, tag="xt", name=f"xt{g}_{kc}")
            nc.sync.dma_start(out=t, in_=xt[kc * 128:(kc + 1) * 128,
                                           g * QG:(g + 1) * QG])
            xt_sb[(g, kc)] = t
            xt_ready[(g, kc)] = dma_est(128 * QG * 2)

    # interleave x(g0) and W_qk chunks so the first qkv m-block can start
    # after one pair instead of after the full weight load; spread the issue
    # across three HWDGE queues (SP/DVE/ACT) since startup is descriptor-
    # generation limited on a single queue
    for kc in range(8):
        t = p_xt.tile([128, QG], BF16, tag="xt", name=f"xt0_{kc}")
        nc.sync.dma_start(out=t, in_=xt[kc * 128:(kc + 1) * 128, 0:QG])
        xt_sb[(0, kc)] = t
        xt_ready[(0, kc)] = dma_est(128 * QG * 2)
        w = p_wqk.tile([128, 2 * CLOC], BF16, tag="wqk", name=f"wqk{kc}")
        nc.sync.dma_start(out=w, in_=wqkt[kc * 128:(kc + 1) * 128, :])
        wqk_sb[kc] = w
        wqk_ready[kc] = dma_est(128 * 2 * CLOC * 2)
    wv_ready = [0.0] * 8
    for kc in range(8):
        w = p_wv.tile([128, CLOC], BF16, tag="wv", name=f"wv{kc}")
        nc.sync.dma_start(out=w, in_=wvt[kc * 128:(kc + 1) * 128, :])
        wv_sb[kc] = w
        wv_ready[kc] = dma_est(128 * CLOC * 2)
    tri2 = p_one.tile([128, 2, KB], BF16, tag="tri2")
    nc.sync.dma_start(out=tri2[:, 0, :], in_=tri[:, :])
    nc.sync.dma_start(out=tri2[:, 1, :], in_=tri[:, :])
    dma_est(2 * KB * KB * 2)
    ones_sb = p_one.tile([1, 64], F32R, tag="ones")
    nc.vector.memset(ones_sb.bitcast(F32), 1.0)
    # preload the Exp table during the initial DMA wait
    scr = p_one.tile([1, 64], F32, tag="scr")
    nc.scalar.activation(out=scr, in_=ones_sb.bitcast(F32), func=EXP, scale=1.0)

    def load_wp():
        for c in range(4):
            w = p_wp.tile([128, C], BF16, tag="wp", name=f"wp{c}")
            nc.sync.dma_start(out=w, in_=wpt[c * 128:(c + 1) * 128, :])
            wp_sb[c] = w
            dma_est(128 * C * 2)

    # persistent attention tensors
    k_sb = [p_k.tile([128, T], BF16, tag="ksb", name=f"ksb{c}") for c in range(4)]
    vp_sb = [None] * 16
    q_sb = {}
    yt_sb = {g: None for g in range(NG)}

    # ---------------- emission helpers ----------------
    pe_t = 0.0
    act_t = 0.0
    q_ready = {}
    v_ready = {}

    def emit_mm_block(g, m, is_v):
        """one qkv m-block: 8 accumulating matmuls + PSUM->SBUF cast"""
        nonlocal pe_t
        ps = ps_mm.tile([128, QG], F32, tag="psmm", name=f"mm{g}_{m}_{is_v}")
        for kc in range(8):
            if is_v:
                nc.tensor.matmul(ps, xt_sb[(g, kc)][:, m * 128:(m + 1) * 128],
                                 wv_sb[kc], start=kc == 0, stop=kc == 7,
                                 skip_group_check=True)
                rdy = max(xt_ready[(g, kc)], wv_ready[kc])
            else:
                nc.tensor.matmul(ps, wqk_sb[kc][:, m * 128:(m + 1) * 128],
                                 xt_sb[(g, kc)], start=kc == 0, stop=kc == 7,
                                 skip_group_check=True)
                rdy = max(xt_ready[(g, kc)], wqk_ready[kc])
            pe_t = max(pe_t + QG * PE_COL, rdy + QG * PE_COL)
        if is_v:
            vp = p_vp.tile([128, HLOC, 65], BF16, tag="vp", name=f"vp{g}_{m}")
            nc.vector.memset(vp[:, :, 64:65], 1.0)
            nc.vector.tensor_copy(out=vp[:, :, 0:64],
                                  in_=ps.rearrange("p (h d) -> p h d", d=64))
            vp_sb[g * 4 + m] = vp
            v_ready[g * 4 + m] = pe_t + CAST_LAT
        elif m >= 4:
            nc.vector.tensor_copy(
                out=k_sb[m - 4][:, g * QG:(g + 1) * QG], in_=ps)
        else:
            qt = p_q.tile([128, QG], BF16, tag="qsb", name=f"q{g}_{m}")
            nc.vector.tensor_copy(out=qt, in_=ps)
            q_sb[(g, m)] = qt
            q_ready[(g, m)] = pe_t + CAST_LAT

    def emit_S(u):
        nonlocal pe_t, act_t
        g, hp, kb = u["g"], u["hp"], u["kb"]
        c0 = u["c0"]
        vis = slice(c0, QG)
        w = QG - c0
        ps = ps_s.tile([128, 2, QG], F32, tag="pss", name=f"s{g}_{hp}_{kb}")
        for r in (0, 1):
            row = slice(64 * r, 64 * r + 64)
            nc.tensor.matmul(ps[:, r, vis],
                             k_sb[hp][row, kb * 128:(kb + 1) * 128],
                             q_sb[(g, hp)][row, vis], start=True, stop=True,
                             skip_group_check=True)
        pe_t = max(pe_t, u["rdy"]) + 2 * w * PE_COL
        es = p_es.tile([128, 2, QG], BF16, tag="es", name=f"e{g}_{hp}_{kb}")
        nc.scalar.activation(out=es[:, :, vis], in_=ps[:, :, vis],
                             func=EXP, scale=SCALE)
        act_t = max(act_t, pe_t + SEM) + 2 * w * ACT_EL + ACT_FIX
        u["es_est"] = act_t + SEM
        if kb >= 4 * g:  # diagonal block: causal tri mask (on idle Pool
            # engine; DVE for the last hp where the tri latency is critical)
            last_hp = g == NG - 1 and hp == 3
            eng = nc.vector if (last_hp or not TRI_POOL) else nc.gpsimd
            eng.tensor_mul(es[:, :, c0:c0 + 128], es[:, :, c0:c0 + 128], tri2)
            u["es_est"] += 250.0 if last_hp else TRI_LAT
        u["es"] = es
        u["exp_done"] = act_t

    def emit_AV(u, psy, k_last):
        nonlocal pe_t
        g, hp, kb = u["g"], u["hp"], u["kb"]
        vis = slice(u["c0"], QG)
        w = QG - u["c0"]
        for r in (0, 1):
            nc.tensor.matmul(psy[0:65, r, vis], vp_sb[kb][:, 2 * hp + r, :],
                             u["es"][:, r, vis], start=u["av_first"],
                             stop=k_last, skip_group_check=True)
        pe_t = max(pe_t, u["av_rdy"]) + 2 * w * PE_COL

    gden = {}
    gysb = {}
    pending_muls = []
    AOP = mybir.AluOpType

    def pop_mul():
        yt, r, ysb, bc = pending_muls.pop(0)
        nc.vector.scalar_tensor_tensor(
            out=yt[64 * r:64 * r + 64, :], in0=ysb[0:64, r, :],
            scalar=-1.0, in1=bc, op0=AOP.mult, op1=AOP.mult)

    def emit_norm(g, hp, psy, pe_bcast):
        """softmax denominators + normalize: PSUM psy -> SBUF yt (bf16).
        Non-tail groups batch 1/denominator as a Newton-seed reciprocal on
        DVE, once per group, keeping the saturated ACT engine exp-only."""
        nonlocal pe_t, act_t
        if yt_sb[g] is None:
            yt_sb[g] = [None] * 4
        ysb = p_ysb.tile([65, 2, QG], F32, tag="ysb", name=f"yb{g}_{hp}")
        if pe_bcast:
            # kernel tail: batched Ln+Exp straight off the PSUM denominator
            # rows; the broadcast matmul then reads the rec rows directly
            yt = p_yt.tile([128, QG], BF16, tag="yt", name=f"yt{g}_{hp}")
            yt_sb[g][hp] = yt
            ln2 = p_one.tile([1, 2, QG], F32, tag="ln2", name=f"ln2{g}_{hp}")
            nc.scalar.activation(out=ln2, in_=psy[64:65, :, :],
                                 func=mybir.ActivationFunctionType.Ln)
            rec2 = p_one.tile([1, 2, QG], F32R, tag="rec2", name=f"rc2{g}_{hp}")
            nc.scalar.activation(out=rec2, in_=ln2, func=EXP, scale=-1.0)
            nc.vector.tensor_copy(out=ysb, in_=psy[0:65, :, :])
            psb = ps_y.tile([128, 2, QG], F32, tag="psy", name=f"pb{g}_{hp}")
            for r in (0, 1):
                nc.tensor.matmul(psb[0:64, r, :], ones_sb, rec2[:, r, :],
                                 start=True, stop=True, skip_group_check=True)
                pe_t += QG * PE_COL
                nc.vector.tensor_mul(yt[64 * r:64 * r + 64, :],
                                     ysb[0:64, r, :], psb[0:64, r, :])
            return
        # single cast frees the PSUM psy tile fast (next hp's AVs wait on
        # it); denominator rows then come from the SBUF copy
        nc.vector.tensor_copy(out=ysb, in_=psy[0:65, :, :])
        if g not in gden:
            gden[g] = p_rec.tile([8, QG], F32, tag="gd", name=f"gd{g}")
            gysb[g] = {}
        gysb[g][hp] = ysb
        # gather both denominator rows with one small DMA (engine writes
        # must start at partition 0/32/64/96; DMA has no such restriction)
        nc.sync.dma_start(out=gden[g][2 * hp:2 * hp + 2, :],
                          in_=ysb[64:65, :, :])
        last_hp = 2 if g == NG - 1 else 3
        if hp != last_hp:
            return
        # one Newton-reciprocal chain for the whole group's 6-8 denominators
        # (sign-carried: y2 = -1/d; the final muls fold in the -1)
        d = gden[g]
        nb = p_rec.tile([8, QG], F32, tag="nb", name=f"nb{g}")
        I32 = mybir.dt.int32
        nc.vector.tensor_tensor(out=nb.bitcast(I32), in0=d.bitcast(I32),
                                in1=d.bitcast(I32), op=AOP.bitwise_not)
        y0 = p_rec.tile([8, QG], F32, tag="y0", name=f"y0{g}")
        nc.vector.tensor_scalar_mul(out=y0, in0=nb, scalar1=0.23549792)
        t1 = p_rec.tile([8, QG], F32, tag="t1", name=f"t1{g}")
        nc.vector.tensor_mul(t1, d, y0)
        y1 = p_rec.tile([8, QG], F32, tag="y1", name=f"y1{g}")
        nc.vector.scalar_tensor_tensor(out=y1, in0=t1, scalar=2.0017324,
                                       in1=y0, op0=AOP.add, op1=AOP.mult)
        t2 = p_rec.tile([8, QG], F32, tag="t2", name=f"t2{g}")
        nc.vector.tensor_mul(t2, d, y1)
        y2 = p_rec.tile([8, QG], F32, tag="y2", name=f"y2{g}")
        nc.vector.scalar_tensor_tensor(out=y2, in0=t2, scalar=2.0,
                                       in1=y1, op0=AOP.add, op1=AOP.mult)
        drec = p_drb.tile([8, QG], F32, tag="drec", name=f"dr{g}")
        nc.sync.dma_start(out=drec, in_=y2)
        for hq in range(last_hp + 1):
            yt = p_yt.tile([128, QG], BF16, tag="yt", name=f"yt{g}_{hq}")
            yt_sb[g][hq] = yt
            for r in (0, 1):
                j = 2 * hq + r
                bc = p_bc.tile([64, QG], F32, tag="bc", name=f"bc{g}_{j}")
                nc.sync.dma_start(
                    out=bc, in_=drec[j:j + 1, :].to_broadcast([64, QG]))
                # defer the mul: the scheduler spreads these through the DVE
                # stream so the 8-op burst doesn't delay qkv/proj PSUM casts
                pending_muls.append((yt, r, gysb[g][hq], bc))

    p3_tiles = {}

    def emit_proj(g, m, c_lo=0, ps=None):
        nonlocal pe_t
        if ps is None:
            ps = ps_mm.tile([128, QG], F32, tag="psmm", name=f"pj{g}_{m}")
        for c in range(c_lo, 4):
            nc.tensor.matmul(ps, wp_sb[c][:, m * 128:(m + 1) * 128],
                             yt_sb[g][c], start=c == 0, stop=c == 3,
                             skip_group_check=True)
        pe_t += (4 - c_lo) * QG * PE_COL
        ost = p_ost.tile([128, QG], BF16, tag="ost", name=f"o{g}_{m}")
        nc.vector.tensor_copy(out=ost, in_=ps)
        # final group's stores go out on the (idle-by-then) ACT queue so the
        # kernel tail doesn't wait behind the SP queue's issue backlog
        eng = nc.scalar if g == NG - 1 else nc.sync
        eng.dma_start(out=ot[m * 128:(m + 1) * 128, g * QG:(g + 1) * QG],
                      in_=ost)

    def emit_proj3_partial(m, ps=None):
        # first 3 contraction chunks of a final-group proj block; the last
        # chunk + store happen in the tail once hp3's yt lands
        nonlocal pe_t
        if ps is None:
            ps = ps_mm.tile([128, QG], F32, tag="psmm", name=f"pj3p_{m}")
        for c in range(3):
            nc.tensor.matmul(ps, wp_sb[c][:, m * 128:(m + 1) * 128],
                             yt_sb[3][c], start=c == 0, stop=False,
                             skip_group_check=True)
        pe_t += 3 * QG * PE_COL
        p3_tiles[m] = ps

    def emit_proj3_partial2(m0):
        # a pair of partial blocks sharing one (by-then idle) S-pool tile
        ps2 = ps_s.tile([128, 2, QG], F32, tag="pss", name=f"pj3q_{m0}")
        emit_proj3_partial(m0, ps2[:, 0, :])
        emit_proj3_partial(m0 + 1, ps2[:, 1, :])

    # ---------------- unit and filler lists ----------------
    # per hp, diagonal key blocks go FIRST: their tri-mask latency then
    # overlaps later units, and the hp's final AV (which releases the psy
    # chain / kernel tail) is an unmasked block
    units = []
    for g in range(NG):
        for hp in range(4):
            order = list(range(4 * g, 4 * (g + 1))) + list(range(4 * g))
            for pos, kb in enumerate(order):
                units.append({"g": g, "hp": hp, "kb": kb,
                              "c0": max(0, 128 * (kb - 4 * g)),
                              "av_first": pos == 0,
                              "av_last": pos == len(order) - 1})
    fillers = []
    for g in range(NG):
        if g > 0:
            fillers.append(("xt", g))
        if g == 1:
            fillers.append(("wp",))
        for hp in range(4):
            fillers.append(("kq", g, 4 + hp))  # k chunk
            fillers.append(("kq", g, hp))      # q chunk
        for tb in range(4):
            fillers.append(("v", g, tb))
    for g in range(NG - 1):
        for m in range(8):
            fillers.append(("proj", g, m))

    # ---------------- greedy clock-driven scheduler ----------------
    s_idx = 0
    f_idx = 0
    av_units = []          # exp-emitted units awaiting AV, lex order
    exp_done_hist = []     # S psum recycle tracking (pool depth 2)
    psy_free_est = 0.0
    yt_ready = {}
    cur_psy = None
    cur_av_key = None      # (g, hp) whose AVs are in flight

    def s_deps(u):
        gk = u["kb"] // 4
        qr = q_ready.get((u["g"], u["hp"]))
        if qr is None or (gk, u["hp"]) not in k_emitted:
            return None
        return max(qr, k_ready.get((gk, u["hp"]), 0.0))

    k_emitted = set()
    k_ready = {}

    def filler_ok(f):
        if f[0] == "proj":
            return f[1] in yt_ready
        if f[0] in ("p3a", "p3b"):
            # pure-tail fill: emit only after the last AV so the partial
            # matmuls don't push the critical-path AVs back in the PE queue
            # (p3b also takes an S-pool tile, unsafe while S units remain)
            return s_idx >= len(units) and not av_units
        return True

    def run_filler(f):
        nonlocal pe_t
        if f[0] == "xt":
            load_xt(f[1])
        elif f[0] == "wp":
            load_wp()
        elif f[0] == "kq":
            g, m = f[1], f[2]
            emit_mm_block(g, m, False)
            if m >= 4:
                k_emitted.add((g, m - 4))
                k_ready[(g, m - 4)] = pe_t + CAST_LAT
        elif f[0] == "v":
            emit_mm_block(f[1], f[2], True)
        elif f[0] == "p3a":
            emit_proj3_partial(f[1])
        elif f[0] == "p3b":
            emit_proj3_partial2(f[1])
        elif f[0] == "proj":
            pe_t = max(pe_t, yt_ready[f[1]])
            emit_proj(f[1], f[2])

    def do_av(u, forced):
        nonlocal pe_t, psy_free_est, cur_av_key, cur_psy
        key = (u["g"], u["hp"])
        if cur_av_key is None:
            if forced:
                pe_t = max(pe_t, psy_free_est)
            cur_psy = ps_y.tile([128, 2, QG], F32, tag="psy",
                                name=f"py{u['g']}_{u['hp']}")
            cur_av_key = key
        av_units.pop(0)
        u["av_rdy"] = max(u["es_est"], v_ready.get(u["kb"], 0.0))
        k_last = u["av_last"]
        emit_AV(u, cur_psy, k_last)
        if k_last:
            g, hp = key
            pe_bcast = g == NG - 1 and hp == 3
            emit_norm(g, hp, cur_psy, pe_bcast)
            psy_free_est = pe_t + NORM_LAT
            if hp == 3:
                yt_ready[g] = pe_t + NORM_LAT + 600.0
            if g == NG - 1 and hp == 2:
                fillers.append(("p3a", 0))
                fillers.append(("p3a", 1))
                fillers.append(("p3b", 2))
                fillers.append(("p3b", 4))
            cur_av_key = None
            cur_psy = None

    def try_S():
        nonlocal s_idx
        u = units[s_idx]
        rdy = s_deps(u)
        depth_ok = (len(exp_done_hist) < 2
                    or exp_done_hist[-2] <= pe_t + 250)
        if (rdy is not None and rdy <= pe_t + 250
                and act_t <= pe_t + LEAD and depth_ok
                and len(av_units) < 15):
            u["rdy"] = rdy
            emit_S(u)
            exp_done_hist.append(u["exp_done"])
            av_units.append(u)
            s_idx += 1
            return True
        return False

    while s_idx < len(units) or av_units or f_idx < len(fillers):
        if pending_muls:
            pop_mul()
        # 1) AV whose es is (estimated) ready
        if av_units:
            u = av_units[0]
            key = (u["g"], u["hp"])
            ok = (u["es_est"] <= pe_t + 200
                  and v_ready.get(u["kb"], 1e18) <= pe_t + 200)
            if ok and cur_av_key is None:
                ok = psy_free_est <= pe_t + 60
            if ok and (cur_av_key is None or cur_av_key == key):
                do_av(u, False)
                continue
        # 2) when ACT already has a healthy backlog, race qkv fillers forward
        # (kq blocks unlock the NEXT group's exp work - emitting them early
        # lets attention pull forward across window boundaries; proj blocks
        # don't enable anything, keep them in reserve for exp-bound gaps)
        act_healthy = act_t > pe_t + KEEP
        if (act_healthy and f_idx < len(fillers)
                and fillers[f_idx][0] != "proj"
                and filler_ok(fillers[f_idx])):
            run_filler(fillers[f_idx])
            f_idx += 1
            continue
        # 3) S unit if deps ready and ACT not over-backlogged
        if s_idx < len(units) and try_S():
            continue
        # 4) filler
        if f_idx < len(fillers) and filler_ok(fillers[f_idx]):
            run_filler(fillers[f_idx])
            f_idx += 1
            continue
        # 5) forced progress (stall): prefer AV, then S, then proj
        if av_units:
            u = av_units[0]
            key = (u["g"], u["hp"])
            if cur_av_key is None or cur_av_key == key:
                do_av(u, True)
                continue
        if s_idx < len(units):
            u = units[s_idx]
            rdy = s_deps(u)
            if rdy is not None:
                pe_t = max(pe_t, rdy)
                if len(exp_done_hist) >= 2:
                    pe_t = max(pe_t, exp_done_hist[-2])
                u["rdy"] = rdy
                emit_S(u)
                exp_done_hist.append(u["exp_done"])
                av_units.append(u)
                s_idx += 1
                continue
        if f_idx < len(fillers):
            f = fillers[f_idx]
            if f[0] == "proj":
                pe_t = max(pe_t, yt_ready.get(f[1], pe_t))
            run_filler(f)
            f_idx += 1
            continue
        raise RuntimeError("scheduler wedged")

    while pending_muls:
        pop_mul()
    # tail: final group's proj (finish the pre-accumulated blocks first)
    for m in range(8):
        if m in p3_tiles:
            emit_proj(NG - 1, m, c_lo=3, ps=p3_tiles[m])
        else:
            emit_proj(NG - 1, m)


def _build_nc():
    from contextlib import ExitStack
    nc = bass.Bass(trn_type="TRN2")
    xt = nc.dram_tensor("xt", [C, T], BF16, kind="ExternalInput")
    wqkt = nc.dram_tensor("wqkt", [C, 2 * CLOC], BF16, kind="ExternalInput")
    wvt = nc.dram_tensor("wvt", [C, CLOC], BF16, kind="ExternalInput")
    wpt = nc.dram_tensor("wpt", [CLOC, C], BF16, kind="ExternalInput")
    tri = nc.dram_tensor("tri", [KB, KB], BF16, kind="ExternalInput")
    ot = nc.dram_tensor("ot", [C, T], BF16, kind="ExternalOutput")
    with tile.TileContext(nc) as tc:
        with ExitStack() as ctx:
            _build_body(nc, tc, ctx, xt, wqkt, wvt, wpt, tri, ot)
    return nc


LAST_RESULTS = None
_NC_CACHE = None


def kernel(x, W_qkv, W_proj):
    global LAST_RESULTS, _NC_CACHE
    import ml_dtypes
    x = np.asarray(x, dtype=np.float32)
    W_qkv = np.asarray(W_qkv, dtype=np.float32)
    W_proj = np.asarray(W_proj, dtype=np.float32)

    if _NC_CACHE is None:
        _NC_CACHE = _build_nc()
    nc = _NC_CACHE
    _conv = lambda a: a.astype(ml_dtypes.bfloat16)
    tri = np.ascontiguousarray(np.triu(np.ones((KB, KB), np.float32)))
    in_maps = []
    for core in range(8):
        b, hg = core // 2, core % 2
        rq = slice(CLOC * hg, CLOC * hg + CLOC)
        Wq = W_qkv[0:C][rq]
        Wk = W_qkv[C:2 * C][rq]
        Wv = W_qkv[2 * C:3 * C][rq]
        in_maps.append({
            "xt": _conv(np.ascontiguousarray(x[b].T)),
            "wqkt": _conv(np.ascontiguousarray(np.concatenate([Wq, Wk], axis=0).T)),
            "wvt": _conv(np.ascontiguousarray(Wv.T)),
            "wpt": _conv(np.ascontiguousarray(W_proj[:, rq].T)),
            "tri": _conv(tri),
        })

    trace = os.environ.get("ATTN_BASS_TRACE") == "1"
    res = None
    last_exc = None
    for attempt in range(3):
        try:
            res = run_bass_kernel_spmd(nc, in_maps, core_ids=list(range(8)),
                                       trace=trace)
            break
        except Exception as e:  # transient NRT device errors happen
            last_exc = e
            import time as _time
            _time.sleep(2.0)
    if res is None:
        raise last_exc
    LAST_RESULTS = res
    out = np.empty((B, T, C), np.float32)
    for b in range(B):
        out[b] = (res.results[2 * b]["ot"].astype(np.float32)
                  + res.results[2 * b + 1]["ot"].astype(np.float32)).T
    return out


# revision 94
# speedup vs baseline: 1.0396x; 1.0106x over previous
"""Trainium2 Bass kernel for NanoAttention (B=4, T=2048, C=1024, H=16, causal).

Sharding: 8 cores = 4 batches x 2 head-groups (8 heads each). Each core:
column-parallel qkv (q,k transposed layout, v natural with an appended ones
column that folds the softmax denominator into the AV matmul), causal
attention over 8 heads, row-parallel proj producing a bf16 [C, T] partial
that the host sums per batch pair.

The builder is a clock-tracking greedy list scheduler. The attention inner
loop is ACT-bound (the exp costs ~2x the S+AV matmul PE time), so attention
units, qkv/proj filler blocks, and normalization chains are interleaved at
emission time to keep both the PE and ACT engines continuously fed (which
also keeps the PE in its fast p-state). Attention for later query groups is
pulled forward into earlier, PE-bound windows as soon as its q/k are ready.

Engine division of labor:
  PE     - all matmuls; tail 1/denom row-broadcast via ones-matmul
  ACT    - softmax exp (the co-bottleneck: kept exp-only) + tail Ln/Exp
  DVE    - PSUM->SBUF casts, group-batched Newton reciprocal for the
           softmax denominators, normalize muls (spread between casts)
  Pool   - causal tri-mask muls (SBUF-only work for an otherwise idle engine)
  DMA    - priority-ordered loads, 1/denom row broadcasts, bf16 stores
           (final group on the ACT queue to dodge the SP issue backlog)

The kernel tail pre-accumulates the final proj blocks' first 3 contraction
chunks during the last head-pair's softmax drip, so only one matmul per
block remains after the last normalization.
"""
import os
import sys

sys.path.insert(0, '/opt/trn_rl_repo')

import numpy as np
import orjson

import concourse.bass as bass
import concourse.mybir as mybir
import concourse.tile as tile
from concourse.bass_utils import run_bass_kernel_spmd

# ---------------------------------------------------------------------------
# Workaround for this container's walrus build: it enforces the HW limit of
# one sync-wait per instruction (two for EventSemaphore), but Tile's sem
# assignment can emit more (kernel-tail Drain waits on every DMA queue used;
# HWDGE stores can pick up two queue waits). Split the overflow onto
# preceding pure-wait EventSemaphore instructions on the same engine at
# JSON-serialization time so every compile path is covered.
# ---------------------------------------------------------------------------


def _split_multi_waits(data):
    n_split = 0
    for func in data.get("functions", []):
        for blk in func.get("blocks", []):
            insts = blk.get("instructions")
            if not insts:
                continue
            out = []
            for inst in insts:
                si = inst.get("sync_info")
                waits = (si or {}).get("on_wait") or []
                cap = 2 if inst.get("opcode") == "EventSemaphore" else 1
                if len(waits) > cap and "engine" in inst:
                    extra = waits[:-cap]
                    si["on_wait"] = waits[-cap:]
                    for i in range(0, len(extra), 2):
                        n_split += 1
                        out.append({
                            "debug": inst.get("debug"),
                            "engine": inst["engine"],
                            "ins": [],
                            "outs": [],
                            "name": f"{inst['name']}_wsplit{n_split}",
                            "opcode": "EventSemaphore",
                            "sync_info": {"on_wait": extra[i:i + 2],
                                          "on_update": []},
                        })
                out.append(inst)
            blk["instructions"] = out
    return data


_orig_to_json_bytes = bass.Bass.to_json_bytes


def _patched_to_json_bytes(self):
    return orjson.dumps(_split_multi_waits(orjson.loads(_orig_to_json_bytes(self))))


bass.Bass.to_json_bytes = _patched_to_json_bytes

# ---------------------------------------------------------------------------

B, T, C = 4, 2048, 1024
N_HEAD, D = 16, 64
HLOC = 8          # heads per core
CLOC = HLOC * D   # 512 local qkv channels per core
QG = 512          # query-group width
NG = T // QG      # 4 query groups
KB = 128          # key-block width
F32R = mybir.dt.float32r
F32 = mybir.dt.float32
BF16 = mybir.dt.bfloat16
EXP = mybir.ActivationFunctionType.Exp
SCALE = 1.0 / np.sqrt(D)
TRI_POOL = os.environ.get("ATTN_TRI", "pool") == "pool"

# scheduler clock model (ns)
PE_COL = 0.43          # per matmul output column, warm clock
ACT_EL = 0.833         # per exp element (per partition-lane)
ACT_FIX = 260.0        # per exp instruction overhead
SEM = 180.0            # cross-engine sem propagation
CAST_LAT = 950.0       # PSUM->SBUF cast completing after producer matmul
TRI_LAT = 480.0        # pool tri-mask mul latency after exp
NORM_LAT = 1600.0      # psy release after last AV (one DVE cast)
LEAD = 6000.0          # how far ACT may run ahead of PE before S throttles
KEEP = 1500.0          # ACT backlog above which fillers take priority over S
DMA_BW = 0.0033        # ns per byte (~300 GB/s effective)
DMA_LAT = 1900.0       # DGE issue + first-byte latency


def _build_body(nc, tc, ctx, xt, wqkt, wvt, wpt, tri, ot):
    p_wqk = ctx.enter_context(tc.tile_pool(name="wqk", bufs=8))
    p_wv = ctx.enter_context(tc.tile_pool(name="wv", bufs=8))
    p_wp = ctx.enter_context(tc.tile_pool(name="wp", bufs=4))
    p_xt = ctx.enter_context(tc.tile_pool(name="xt", bufs=16))
    p_k = ctx.enter_context(tc.tile_pool(name="ksb", bufs=4))
    p_q = ctx.enter_context(tc.tile_pool(name="qsb", bufs=8))
    p_vp = ctx.enter_context(tc.tile_pool(name="vp", bufs=16))
    p_es = ctx.enter_context(tc.tile_pool(name="es", bufs=16))
    p_ysb = ctx.enter_context(tc.tile_pool(name="ysb", bufs=5))
    p_rec = ctx.enter_context(tc.tile_pool(name="rec", bufs=1))
    p_bc = ctx.enter_context(tc.tile_pool(name="bc", bufs=8))
    p_yt = ctx.enter_context(tc.tile_pool(name="yt", bufs=16))
    p_ost = ctx.enter_context(tc.tile_pool(name="ost", bufs=4))
    p_one = ctx.enter_context(tc.tile_pool(name="one", bufs=1))
    p_drb = ctx.enter_context(tc.tile_pool(name="drb", bufs=4, space="DRAM"))
    ps_mm = ctx.enter_context(tc.tile_pool(name="psmm", bufs=2, space="PSUM"))
    ps_s = ctx.enter_context(tc.tile_pool(name="pss", bufs=2, space="PSUM"))
    ps_y = ctx.enter_context(tc.tile_pool(name="psy", bufs=1, space="PSUM"))

    # ---------------- static tensors / initial DMA batch ----------------
    dma_t = 0.0

    def dma_est(nbytes):
        nonlocal dma_t
        dma_t += nbytes * DMA_BW
        return dma_t + DMA_LAT

    wqk_sb = [None] * 8
    wv_sb = [None] * 8
    wp_sb = [None] * 4
    xt_sb = {}
    xt_ready = {}
    wqk_ready = [0.0] * 8

    def load_xt(g):
        for kc in range(8):
            t = p_xt.tile([128, QG], BF16, tag="xt", name=f"xt{g}_{kc}")
            nc.sync.dma_start(out=t, in_=xt[kc * 128:(kc + 1) * 128,
                                           g * QG:(g + 1) * QG])
            xt_sb[(g, kc)] = t
            xt_ready[(g, kc)] = dma_est(128 * QG * 2)

    # interleave x(g0) and W_qk chunks so the first qkv m-block can start
    # after one pair instead of after the full weight load; spread the issue
    # across three HWDGE queues (SP/DVE/ACT) since startup is descriptor-
    # generation limited on a single queue
    for kc in range(8):
        t = p_xt.tile([128, QG], BF16, tag="xt", name=f"xt0_{kc}")
        nc.sync.dma_start(out=t, in_=xt[kc * 128:(kc + 1) * 128, 0:QG])
        xt_sb[(0, kc)] = t
        xt_ready[(0, kc)] = dma_est(128 * QG * 2)
        w = p_wqk.tile([128, 2 * CLOC], BF16, tag="wqk", name=f"wqk{kc}")
        nc.sync.dma_start(out=w, in_=wqkt[kc * 128:(kc + 1) * 128, :])
        wqk_sb[kc] = w
        wqk_ready[kc] = dma_est(128 * 2 * CLOC * 2)
    wv_ready = [0.0] * 8
    for kc in range(8):
        w = p_wv.tile([128, CLOC], BF16, tag="wv", name=f"wv{kc}")
        nc.sync.dma_start(out=w, in_=wvt[kc * 128:(kc + 1) * 128, :])
        wv_sb[kc] = w
        wv_ready[kc] = dma_est(128 * CLOC * 2)
    tri2 = p_one.tile([128, 2, KB], BF16, tag="tri2")
    nc.sync.dma_start(out=tri2[:, 0, :], in_=tri[:, :])
    nc.sync.dma_start(out=tri2[:, 1, :], in_=tri[:, :])
    dma_est(2 * KB * KB * 2)
    ones_sb = p_one.tile([1, 64], F32R, tag="ones")
    nc.vector.memset(ones_sb.bitcast(F32), 1.0)
    # preload the Exp table during the initial DMA wait
    scr = p_one.tile([1, 64], F32, tag="scr")
    nc.scalar.activation(out=scr, in_=ones_sb.bitcast(F32), func=EXP, scale=1.0)

    def load_wp():
        for c in range(4):
            w = p_wp.tile([128, C], BF16, tag="wp", name=f"wp{c}")
            nc.sync.dma_start(out=w, in_=wpt[c * 128:(c + 1) * 128, :])
            wp_sb[c] = w
            dma_est(128 * C * 2)

    # persistent attention tensors
    k_sb = [p_k.tile([128, T], BF16, tag="ksb", name=f"ksb{c}") for c in range(4)]
    vp_sb = [None] * 16
    q_sb = {}
    yt_sb = {g: None for g in range(NG)}

    # ---------------- emission helpers ----------------
    pe_t = 0.0
    act_t = 0.0
    q_ready = {}
    v_ready = {}

    def emit_mm_block(g, m, is_v):
        """one qkv m-block: 8 accumulating matmuls + PSUM->SBUF cast"""
        nonlocal pe_t
        ps = ps_mm.tile([128, QG], F32, tag="psmm", name=f"mm{g}_{m}_{is_v}")
        for kc in range(8):
            if is_v:
                nc.tensor.matmul(ps, xt_sb[(g, kc)][:, m * 128:(m + 1) * 128],
                                 wv_sb[kc], start=kc == 0, stop=kc == 7,
                                 skip_group_check=True)
                rdy = max(xt_ready[(g, kc)], wv_ready[kc])
            else:
                nc.tensor.matmul(ps, wqk_sb[kc][:, m * 128:(m + 1) * 128],
                                 xt_sb[(g, kc)], start=kc == 0, stop=kc == 7,
                                 skip_group_check=True)
                rdy = max(xt_ready[(g, kc)], wqk_ready[kc])
            pe_t = max(pe_t + QG * PE_COL, rdy + QG * PE_COL)
        if is_v:
            vp = p_vp.tile([128, HLOC, 65], BF16, tag="vp", name=f"vp{g}_{m}")
            nc.vector.memset(vp[:, :, 64:65], 1.0)
            nc.vector.tensor_copy(out=vp[:, :, 0:64],
                                  in_=ps.rearrange("p (h d) -> p h d", d=64))
            vp_sb[g * 4 + m] = vp
            v_ready[g * 4 + m] = pe_t + CAST_LAT
        elif m >= 4:
            nc.vector.tensor_copy(
                out=k_sb[m - 4][:, g * QG:(g + 1) * QG], in_=ps)
        else:
            qt = p_q.tile([128, QG], BF16, tag="qsb", name=f"q{g}_{m}")
            nc.vector.tensor_copy(out=qt, in_=ps)
            q_sb[(g, m)] = qt
            q_ready[(g, m)] = pe_t + CAST_LAT

    def emit_S(u):
        nonlocal pe_t, act_t
        g, hp, kb = u["g"], u["hp"], u["kb"]
        c0 = u["c0"]
        vis = slice(c0, QG)
        w = QG - c0
        ps = ps_s.tile([128, 2, QG], F32, tag="pss", name=f"s{g}_{hp}_{kb}")
        for r in (0, 1):
            row = slice(64 * r, 64 * r + 64)
            nc.tensor.matmul(ps[:, r, vis],
                             k_sb[hp][row, kb * 128:(kb + 1) * 128],
                             q_sb[(g, hp)][row, vis], start=True, stop=True,
                             skip_group_check=True)
        pe_t = max(pe_t, u["rdy"]) + 2 * w * PE_COL
        es = p_es.tile([128, 2, QG], BF16, tag="es", name=f"e{g}_{hp}_{kb}")
        nc.scalar.activation(out=es[:, :, vis], in_=ps[:, :, vis],
                             func=EXP, scale=SCALE)
        act_t = max(act_t, pe_t + SEM) + 2 * w * ACT_EL + ACT_FIX
        u["es_est"] = act_t + SEM
        if kb >= 4 * g:  # diagonal block: causal tri mask (on idle Pool
            # engine; DVE for the last hp where the tri latency is critical)
            last_hp = g == NG - 1 and hp == 3
            eng = nc.vector if (last_hp or not TRI_POOL) else nc.gpsimd
            eng.tensor_mul(es[:, :, c0:c0 + 128], es[:, :, c0:c0 + 128], tri2)
            u["es_est"] += 250.0 if last_hp else TRI_LAT
        u["es"] = es
        u["exp_done"] = act_t

    def emit_AV(u, psy, k_last):
        nonlocal pe_t
        g, hp, kb = u["g"], u["hp"], u["kb"]
        vis = slice(u["c0"], QG)
        w = QG - u["c0"]
        for r in (0, 1):
            nc.tensor.matmul(psy[0:65, r, vis], vp_sb[kb][:, 2 * hp + r, :],
                             u["es"][:, r, vis], start=u["av_first"],
                             stop=k_last, skip_group_check=True)
        pe_t = max(pe_t, u["av_rdy"]) + 2 * w * PE_COL

    gden = {}
    gysb = {}
    pending_muls = []
    AOP = mybir.AluOpType

    def pop_mul():
        yt, r, ysb, bc = pending_muls.pop(0)
        nc.vector.scalar_tensor_tensor(
            out=yt[64 * r:64 * r + 64, :], in0=ysb[0:64, r, :],
            scalar=-1.0, in1=bc, op0=AOP.mult, op1=AOP.mult)

    def emit_norm(g, hp, psy, pe_bcast):
        """softmax denominators + normalize: PSUM psy -> SBUF yt (bf16).
        Non-tail groups batch 1/denominator as a Newton-seed reciprocal on
        DVE, once per group, keeping the saturated ACT engine exp-only."""
        nonlocal pe_t, act_t
        if yt_sb[g] is None:
            yt_sb[g] = [None] * 4
        ysb = p_ysb.tile([65, 2, QG], F32, tag="ysb", name=f"yb{g}_{hp}")
        if pe_bcast:
            # kernel tail: batched Ln+Exp straight off the PSUM denominator
            # rows; the broadcast matmul then reads the rec rows directly
            yt = p_yt.tile([128, QG], BF16, tag="yt", name=f"yt{g}_{hp}")
            yt_sb[g][hp] = yt
            ln2 = p_one.tile([1, 2, QG], F32, tag="ln2", name=f"ln2{g}_{hp}")
            nc.scalar.activation(out=ln2, in_=psy[64:65, :, :],
                                 func=mybir.ActivationFunctionType.Ln)
            rec2 = p_one.tile([1, 2, QG], F32R, tag="rec2", name=f"rc2{g}_{hp}")
            nc.scalar.activation(out=rec2, in_=ln2, func=EXP, scale=-1.0)
            nc.vector.tensor_copy(out=ysb, in_=psy[0:65, :, :])
            psb = ps_y.tile([128, 2, QG], F32, tag="psy", name=f"pb{g}_{hp}")
            for r in (0, 1):
                nc.tensor.matmul(psb[0:64, r, :], ones_sb, rec2[:, r, :],
                                 start=True, stop=True, skip_group_check=True)
                pe_t += QG * PE_COL
                nc.vector.tensor_mul(yt[64 * r:64 * r + 64, :],
                                     ysb[0:64, r, :], psb[0:64, r, :])
            return
        # single cast frees the PSUM psy tile fast (next hp's AVs wait on
        # it); denominator rows then come from the SBUF copy
        nc.vector.tensor_copy(out=ysb, in_=psy[0:65, :, :])
        if g not in gden:
            gden[g] = p_rec.tile([8, QG], F32, tag="gd", name=f"gd{g}")
            gysb[g] = {}
        gysb[g][hp] = ysb
        # gather both denominator rows with one small DMA (engine writes
        # must start at partition 0/32/64/96; DMA has no such restriction)
        nc.sync.dma_start(out=gden[g][2 * hp:2 * hp + 2, :],
                          in_=ysb[64:65, :, :])
        last_hp = 2 if g == NG - 1 else 3
        if hp != last_hp:
            return
        # one Newton-reciprocal chain for the whole group's 6-8 denominators
        # (sign-carried: y2 = -1/d; the final muls fold in the -1)
        d = gden[g]
        nb = p_rec.tile([8, QG], F32, tag="nb", name=f"nb{g}")
        I32 = mybir.dt.int32
        nc.vector.tensor_tensor(out=nb.bitcast(I32), in0=d.bitcast(I32),
                                in1=d.bitcast(I32), op=AOP.bitwise_not)
        y0 = p_rec.tile([8, QG], F32, tag="y0", name=f"y0{g}")
        nc.vector.tensor_scalar_mul(out=y0, in0=nb, scalar1=0.23549792)
        t1 = p_rec.tile([8, QG], F32, tag="t1", name=f"t1{g}")
        nc.vector.tensor_mul(t1, d, y0)
        y1 = p_rec.tile([8, QG], F32, tag="y1", name=f"y1{g}")
        nc.vector.scalar_tensor_tensor(out=y1, in0=t1, scalar=2.0017324,
                                       in1=y0, op0=AOP.add, op1=AOP.mult)
        t2 = p_rec.tile([8, QG], F32, tag="t2", name=f"t2{g}")
        nc.vector.tensor_mul(t2, d, y1)
        y2 = p_rec.tile([8, QG], F32, tag="y2", name=f"y2{g}")
        nc.vector.scalar_tensor_tensor(out=y2, in0=t2, scalar=2.0,
                                       in1=y1, op0=AOP.add, op1=AOP.mult)
        drec = p_drb.tile([8, QG], F32, tag="drec", name=f"dr{g}")
        nc.sync.dma_start(out=drec, in_=y2)
        for hq in range(last_hp + 1):
            yt = p_yt.tile([128, QG], BF16, tag="yt", name=f"yt{g}_{hq}")
            yt_sb[g][hq] = yt
            for r in (0, 1):
                j = 2 * hq + r
                bc = p_bc.tile([64, QG], F32, tag="bc", name=f"bc{g}_{j}")
                nc.sync.dma_start(
                    out=bc, in_=drec[j:j + 1, :].to_broadcast([64, QG]))
                # defer the mul: the scheduler spreads these through the DVE
                # stream so the 8-op burst doesn't delay qkv/proj PSUM casts
                pending_muls.append((yt, r, gysb[g][hq], bc))

    p3_tiles = {}

    def emit_proj(g, m, c_lo=0, ps=None):
        nonlocal pe_t
        if ps is None:
            ps = ps_mm.tile([128, QG], F32, tag="psmm", name=f"pj{g}_{m}")
        for c in range(c_lo, 4):
            nc.tensor.matmul(ps, wp_sb[c][:, m * 128:(m + 1) * 128],
                             yt_sb[g][c], start=c == 0, stop=c == 3,
                             skip_group_check=True)
        pe_t += (4 - c_lo) * QG * PE_COL
        ost = p_ost.tile([128, QG], BF16, tag="ost", name=f"o{g}_{m}")
        nc.vector.tensor_copy(out=ost, in_=ps)
        # final group's stores go out on the (idle-by-then) ACT queue so the
        # kernel tail doesn't wait behind the SP queue's issue backlog
        eng = nc.scalar if g == NG - 1 else nc.sync
        eng.dma_start(out=ot[m * 128:(m + 1) * 128, g * QG:(g + 1) * QG],
                      in_=ost)

    def emit_proj3_partial(m, ps=None):
        # first 3 contraction chunks of a final-group proj block; the last
        # chunk + store happen in the tail once hp3's yt lands
        nonlocal pe_t
        if ps is None:
            ps = ps_mm.tile([128, QG], F32, tag="psmm", name=f"pj3p_{m}")
        for c in range(3):
            nc.tensor.matmul(ps, wp_sb[c][:, m * 128:(m + 1) * 128],
                             yt_sb[3][c], start=c == 0, stop=False,
                             skip_group_check=True)
        pe_t += 3 * QG * PE_COL
        p3_tiles[m] = ps

    def emit_proj3_partial2(m0):
        # a pair of partial blocks sharing one (by-then idle) S-pool tile
        ps2 = ps_s.tile([128, 2, QG], F32, tag="pss", name=f"pj3q_{m0}")
        emit_proj3_partial(m0, ps2[:, 0, :])
        emit_proj3_partial(m0 + 1, ps2[:, 1, :])

    # ---------------- unit and filler lists ----------------
    # per hp, diagonal key blocks go FIRST: their tri-mask latency then
    # overlaps later units, and the hp's final AV (which releases the psy
    # chain / kernel tail) is an unmasked block
    units = []
    for g in range(NG):
        for hp in range(4):
            order = list(range(4 * g, 4 * (g + 1))) + list(range(4 * g))
            for pos, kb in enumerate(order):
                units.append({"g": g, "hp": hp, "kb": kb,
                              "c0": max(0, 128 * (kb - 4 * g)),
                              "av_first": pos == 0,
                              "av_last": pos == len(order) - 1})
    fillers = []
    for g in range(NG):
        if g > 0:
            fillers.append(("xt", g))
        if g == 1:
            fillers.append(("wp",))
        for hp in range(4):
            fillers.append(("kq", g, 4 + hp))  # k chunk
            fillers.append(("kq", g, hp))      # q chunk
        for tb in range(4):
            fillers.append(("v", g, tb))
    for g in range(NG - 1):
        for m in range(8):
            fillers.append(("proj", g, m))

    # ---------------- greedy clock-driven scheduler ----------------
    s_idx = 0
    f_idx = 0
    av_units = []          # exp-emitted units awaiting AV, lex order
    exp_done_hist = []     # S psum recycle tracking (pool depth 2)
    psy_free_est = 0.0
    yt_ready = {}
    cur_psy = None
    cur_av_key = None      # (g, hp) whose AVs are in flight

    def s_deps(u):
        gk = u["kb"] // 4
        qr = q_ready.get((u["g"], u["hp"]))
        if qr is None or (gk, u["hp"]) not in k_emitted:
            return None
        return max(qr, k_ready.get((gk, u["hp"]), 0.0))

    k_emitted = set()
    k_ready = {}

    def filler_ok(f):
        if f[0] == "proj":
            return f[1] in yt_ready
        if f[0] in ("p3a", "p3b"):
            # pure-tail fill: emit only after the last AV so the partial
            # matmuls don't push the critical-path AVs back in the PE queue
            # (p3b also takes an S-pool tile, unsafe while S units remain)
            return s_idx >= len(units) and not av_units
        return True

    def run_filler(f):
        nonlocal pe_t
        if f[0] == "xt":
            load_xt(f[1])
        elif f[0] == "wp":
            load_wp()
        elif f[0] == "kq":
            g, m = f[1], f[2]
            emit_mm_block(g, m, False)
            if m >= 4:
                k_emitted.add((g, m - 4))
                k_ready[(g, m - 4)] = pe_t + CAST_LAT
        elif f[0] == "v":
            emit_mm_block(f[1], f[2], True)
        elif f[0] == "p3a":
            emit_proj3_partial(f[1])
        elif f[0] == "p3b":
            emit_proj3_partial2(f[1])
        elif f[0] == "proj":
            pe_t = max(pe_t, yt_ready[f[1]])
            emit_proj(f[1], f[2])

    def do_av(u, forced):
        nonlocal pe_t, psy_free_est, cur_av_key, cur_psy
        key = (u["g"], u["hp"])
        if cur_av_key is None:
            if forced:
                pe_t = max(pe_t, psy_free_est)
            cur_psy = ps_y.tile([128, 2, QG], F32, tag="psy",
                                name=f"py{u['g']}_{u['hp']}")
            cur_av_key = key
        av_units.pop(0)
        u["av_rdy"] = max(u["es_est"], v_ready.get(u["kb"], 0.0))
        k_last = u["av_last"]
        emit_AV(u, cur_psy, k_last)
        if k_last:
            g, hp = key
            pe_bcast = g == NG - 1 and hp == 3
            emit_norm(g, hp, cur_psy, pe_bcast)
            psy_free_est = pe_t + NORM_LAT
            if hp == 3:
                yt_ready[g] = pe_t + NORM_LAT + 600.0
            if g == NG - 1 and hp == 2:
                fillers.append(("p3a", 0))
                fillers.append(("p3a", 1))
                fillers.append(("p3b", 2))
                fillers.append(("p3b", 4))
            cur_av_key = None
            cur_psy = None

    def try_S():
        nonlocal s_idx
        u = units[s_idx]
        rdy = s_deps(u)
        depth_ok = (len(exp_done_hist) < 2
                    or exp_done_hist[-2] <= pe_t + 380)
        if (rdy is not None and rdy <= pe_t + 380
                and act_t <= pe_t + LEAD and depth_ok
                and len(av_units) < 15):
            u["rdy"] = rdy
            emit_S(u)
            exp_done_hist.append(u["exp_done"])
            av_units.append(u)
            s_idx += 1
            return True
        return False

    while s_idx < len(units) or av_units or f_idx < len(fillers):
        if pending_muls:
            pop_mul()
        # 1) AV whose es is (estimated) ready
        if av_units:
            u = av_units[0]
            key = (u["g"], u["hp"])
            ok = (u["es_est"] <= pe_t + 380
                  and v_ready.get(u["kb"], 1e18) <= pe_t + 380)
            if ok and cur_av_key is None:
                ok = psy_free_est <= pe_t + 60
            if ok and (cur_av_key is None or cur_av_key == key):
                do_av(u, False)
                continue
        # 2) when ACT already has a healthy backlog, race qkv fillers forward
        # (kq blocks unlock the NEXT group's exp work - emitting them early
        # lets attention pull forward across window boundaries; proj blocks
        # don't enable anything, keep them in reserve for exp-bound gaps)
        act_healthy = act_t > pe_t + KEEP
        if (act_healthy and f_idx < len(fillers)
                and fillers[f_idx][0] != "proj"
                and filler_ok(fillers[f_idx])):
            run_filler(fillers[f_idx])
            f_idx += 1
            continue
        # 3) S unit if deps ready and ACT not over-backlogged
        if s_idx < len(units) and try_S():
            continue
        # 4) filler
        if f_idx < len(fillers) and filler_ok(fillers[f_idx]):
            run_filler(fillers[f_idx])
            f_idx += 1
            continue
        # 5) forced progress (stall): prefer AV, then S, then proj
        if av_units:
            u = av_units[0]
            key = (u["g"], u["hp"])
            if cur_av_key is None or cur_av_key == key:
                do_av(u, True)
                continue
        if s_idx < len(units):
            u = units[s_idx]
            rdy = s_deps(u)
            if rdy is not None:
                pe_t = max(pe_t, rdy)
                if len(exp_done_hist) >= 2:
                    pe_t = max(pe_t, exp_done_hist[-2])
                u["rdy"] = rdy
                emit_S(u)
                exp_done_hist.append(u["exp_done"])
                av_units.append(u)
                s_idx += 1
                continue
        if f_idx < len(fillers):
            f = fillers[f_idx]
            if f[0] == "proj":
                pe_t = max(pe_t, yt_ready.get(f[1], pe_t))
            run_filler(f)
            f_idx += 1
            continue
        raise RuntimeError("scheduler wedged")

    while pending_muls:
        pop_mul()
    # tail: final group's proj (finish the pre-accumulated blocks first)
    for m in range(8):
        if m in p3_tiles:
            emit_proj(NG - 1, m, c_lo=3, ps=p3_tiles[m])
        else:
            emit_proj(NG - 1, m)


def _build_nc():
    from contextlib import ExitStack
    nc = bass.Bass(trn_type="TRN2")
    xt = nc.dram_tensor("xt", [C, T], BF16, kind="ExternalInput")
    wqkt = nc.dram_tensor("wqkt", [C, 2 * CLOC], BF16, kind="ExternalInput")
    wvt = nc.dram_tensor("wvt", [C, CLOC], BF16, kind="ExternalInput")
    wpt = nc.dram_tensor("wpt", [CLOC, C], BF16, kind="ExternalInput")
    tri = nc.dram_tensor("tri", [KB, KB], BF16, kind="ExternalInput")
    ot = nc.dram_tensor("ot", [C, T], BF16, kind="ExternalOutput")
    with tile.TileContext(nc) as tc:
        with ExitStack() as ctx:
            _build_body(nc, tc, ctx, xt, wqkt, wvt, wpt, tri, ot)
    return nc


LAST_RESULTS = None
_NC_CACHE = None


def kernel(x, W_qkv, W_proj):
    global LAST_RESULTS, _NC_CACHE
    import ml_dtypes
    x = np.asarray(x, dtype=np.float32)
    W_qkv = np.asarray(W_qkv, dtype=np.float32)
    W_proj = np.asarray(W_proj, dtype=np.float32)

    if _NC_CACHE is None:
        _NC_CACHE = _build_nc()
    nc = _NC_CACHE
    _conv = lambda a: a.astype(ml_dtypes.bfloat16)
    tri = np.ascontiguousarray(np.triu(np.ones((KB, KB), np.float32)))
    in_maps = []
    for core in range(8):
        b, hg = core // 2, core % 2
        rq = slice(CLOC * hg, CLOC * hg + CLOC)
        Wq = W_qkv[0:C][rq]
        Wk = W_qkv[C:2 * C][rq]
        Wv = W_qkv[2 * C:3 * C][rq]
        in_maps.append({
            "xt": _conv(np.ascontiguousarray(x[b].T)),
            "wqkt": _conv(np.ascontiguousarray(np.concatenate([Wq, Wk], axis=0).T)),
            "wvt": _conv(np.ascontiguousarray(Wv.T)),
            "wpt": _conv(np.ascontiguousarray(W_proj[:, rq].T)),
            "tri": _conv(tri),
        })

    trace = os.environ.get("ATTN_BASS_TRACE") == "1"
    res = None
    last_exc = None
    for attempt in range(3):
        try:
            res = run_bass_kernel_spmd(nc, in_maps, core_ids=list(range(8)),
                                       trace=trace)
            break
        except Exception as e:  # transient NRT device errors happen
            last_exc = e
            import time as _time
            _time.sleep(2.0)
    if res is None:
        raise last_exc
    LAST_RESULTS = res
    out = np.empty((B, T, C), np.float32)
    for b in range(B):
        out[b] = (res.results[2 * b]["ot"].astype(np.float32)
                  + res.results[2 * b + 1]["ot"].astype(np.float32)).T
    return out
